# revision 54
# baseline (speedup 1.0000x reference)
"""Trainium2 Bass kernel for nn_AdaptiveMBlock (three rank-select pools + 1x1 conv).

Self-contained: the selection-network schedule is baked in below.

Strategy:
  - 8 cores = (batch 4) x (H halves of 128 rows).  Host pads H and W by 3
    with zeros (SAME padding), casts to bf16, ships each core a
    [134, 262*16] shard.
  - On-chip layout: partitions = 128 H rows, free dim = (W, C) with C
    contiguous.  Vertical window offsets come from 7 row-shifted DMA loads;
    horizontal offsets are free-dim access-pattern offsets (free).
  - Column stage: shared sorting networks produce sorted 3/5/7-tall column
    planes; horizontal stage: hash-consed pruned odd-even merge networks
    select rank 6/9, 14/25, 26/49 per pixel.  All ops are DVE
    tensor_tensor min/max in bf16 (2x perf mode).
  - 1x1 conv (48->16) on the TensorEngine via per-128-block transposes and
    block-diagonal weights; bf16 result written back transposed into PSUM
    and DMAed straight from PSUM; host casts to f32.
  - Raw bass (no Tile): explicit per-engine programs with counting
    semaphores, each instruction carrying at most one wait (this
    container's walrus rejects multi-wait instructions).
  - Two 128-wide W chunks.  Chunk-0 leaf loads are staged in four waves
    split across DMA queues so DVE starts early.  m-planes complete in
    order m3 < m5 < m7; the conv consumes planes in that order via three
    per-plane semaphores so transposes/copies overlap the tail.
  - All value planes (leaves, slots, m-planes) live in ONE SBUF slab;
    independent same-op same-width schedule nodes are fused pairwise into
    single DVE instructions via 3D offset+stride APs (~40% fewer DVE
    instructions; saves the ~150ns per-instruction overhead).  The slab
    slot order is an offline local-search result maximizing pairs whose
    AP strides fit the signed-16-bit ISA field.
  - The final m7 op is emitted in four 32-col quarters incrementing the
    plane semaphore each, so the conv tail starts ~4 transposes earlier.
    (NOTE: partial-count waits on DMA wave semaphores are UNSAFE — piece
    completions across the 16 DMA engines are unordered; only full-wave
    counts may be waited on.)
"""
import os
import sys
from contextlib import ExitStack

sys.path.insert(0, "/opt/trn_rl_repo")

import numpy as np
import ml_dtypes

import concourse.bass as bass
import concourse.mybir as mybir
from concourse.ap import AP as BassAP
from concourse.bass_utils import run_bass_kernel_spmd

# ---- baked selection-network schedule ----
# Auto-generated by bake_schedule.py — selection network schedule
LEAVES = [-3, -2, -1, 0, 1, 2, 3]
NODES = [('min', 2, 0, 3, 0), ('max', 2, 0, 3, 0), ('min', 4, 0, 8, 0), ('max', 4, 0, 8, 0), ('min', 7, 0, 9, 0), ('max', 7, 0, 9, 0), ('min', 1, 0, 5, 0), ('max', 1, 0, 5, 0), ('min', 11, 0, 13, 0), ('max', 11, 0, 13, 0), ('min', 10, 0, 16, 0), ('max', 10, 0, 16, 0), ('min', 12, 0, 14, 0), ('max', 12, 0, 14, 0), ('min', 17, 0, 19, 0), ('max', 17, 0, 19, 0), ('min', 18, 0, 20, 0), ('max', 18, 0, 20, 0), ('min', 0, 0, 6, 0), ('max', 0, 0, 6, 0), ('min', 15, 0, 25, 0), ('max', 15, 0, 25, 0), ('min', 24, 0, 28, 0), ('max', 24, 0, 28, 0), ('min', 22, 0, 29, 0), ('max', 22, 0, 29, 0), ('min', 21, 0, 26, 0), ('max', 21, 0, 26, 0), ('min', 23, 0, 34, 0), ('max', 23, 0, 34, 0), ('min', 31, 0, 33, 0), ('max', 31, 0, 33, 0), ('min', 32, 0, 35, 0), ('max', 32, 0, 35, 0), ('min', 30, 0, 36, 0), ('max', 30, 0, 36, 0), ('max', 11, 0, 11, -1), ('min', 10, 0, 10, -1), ('max', 10, 0, 10, -1), ('max', 43, 0, 44, 0), ('max', 12, 0, 12, -1), ('min', 46, 0, 47, 0), ('max', 46, 0, 47, 0), ('max', 10, 0, 48, -1), ('max', 12, 0, 49, -1), ('max', 11, 0, 45, -1), ('min', 50, 0, 51, 0), ('min', 52, 0, 53, 0), ('min', 15, 0, 15, -1), ('max', 15, 0, 15, -1), ('min', 24, 0, 24, -1), ('max', 24, 0, 24, -1), ('min', 56, 0, 57, 0), ('max', 56, 0, 57, 0), ('min', 22, 0, 22, -1), ('max', 22, 0, 22, -1), ('min', 59, 0, 61, 0), ('max', 59, 0, 61, 0), ('min', 60, 0, 62, 0), ('max', 60, 0, 62, 0), ('min', 21, 0, 21, -1), ('max', 21, 0, 21, -1), ('min', 23, 0, 23, -1), ('max', 23, 0, 23, -1), ('min', 68, 0, 69, 0), ('max', 68, 0, 69, 0), ('min', 63, 0, 67, 0), ('max', 63, 0, 67, 0), ('min', 64, 0, 71, 0), ('max', 64, 0, 71, 0), ('min', 65, 0, 72, 0), ('max', 65, 0, 72, 0), ('min', 66, 0, 70, 0), ('max', 66, 0, 70, 0), ('max', 55, 0, 55, -2), ('min', 80, 0, 80, -2), ('max', 81, 0, 82, 0), ('max', 76, 0, 76, -2), ('min', 83, 0, 84, 0), ('max', 83, 0, 84, 0), ('max', 74, 0, 74, -2), ('min', 78, 0, 78, -2), ('max', 78, 0, 78, -2), ('max', 87, 0, 88, 0), ('min', 85, 0, 90, 0), ('max', 85, 0, 90, 0), ('min', 86, 0, 89, 0), ('max', 73, 0, 73, -2), ('min', 58, 0, 58, -2), ('min', 94, 0, 95, 0), ('max', 94, 0, 95, 0), ('min', 77, 0, 77, -2), ('max', 77, 0, 77, -2), ('max', 96, 0, 98, 0), ('min', 97, 0, 99, 0), ('max', 75, 0, 75, -2), ('min', 79, 0, 79, -2), ('min', 102, 0, 103, 0), ('max', 102, 0, 103, 0), ('max', 100, 0, 104, 0), ('min', 101, 0, 105, 0), ('max', 101, 0, 105, 0), ('min', 91, 0, 106, 0), ('max', 91, 0, 106, 0), ('min', 92, 0, 107, 0), ('max', 92, 0, 107, 0), ('min', 93, 0, 108, 0), ('max', 93, 0, 108, 0), ('max', 24, 0, 109, -1), ('max', 23, 0, 110, -1), ('max', 22, 0, 111, -1), ('max', 21, 0, 112, -1), ('max', 15, 0, 113, -1), ('min', 115, 0, 116, 0), ('min', 117, 0, 118, 0), ('min', 114, 0, 119, 1), ('min', 120, 0, 121, 0), ('min', 122, 0, 123, 1), ('min', 27, 0, 27, -1), ('max', 27, 0, 27, -1), ('min', 40, 0, 40, -1), ('max', 40, 0, 40, -1), ('min', 126, 0, 127, 0), ('max', 126, 0, 127, 0), ('min', 38, 0, 38, -1), ('max', 38, 0, 38, -1), ('min', 42, 0, 42, -1), ('max', 42, 0, 42, -1), ('min', 132, 0, 133, 0), ('max', 132, 0, 133, 0), ('min', 129, 0, 131, 0), ('max', 129, 0, 131, 0), ('min', 130, 0, 135, 0), ('max', 130, 0, 135, 0), ('min', 128, 0, 136, 0), ('max', 128, 0, 136, 0), ('min', 37, 0, 37, -1), ('max', 37, 0, 37, -1), ('min', 41, 0, 41, -1), ('max', 41, 0, 41, -1), ('min', 144, 0, 145, 0), ('max', 144, 0, 145, 0), ('min', 39, 0, 39, -1), ('max', 39, 0, 39, -1), ('min', 147, 0, 149, 0), ('max', 147, 0, 149, 0), ('min', 148, 0, 150, 0), ('max', 148, 0, 150, 0), ('min', 137, 0, 143, 0), ('max', 137, 0, 143, 0), ('min', 138, 0, 151, 0), ('max', 138, 0, 151, 0), ('min', 139, 0, 152, 0), ('max', 139, 0, 152, 0), ('min', 140, 0, 153, 0), ('max', 140, 0, 153, 0), ('min', 141, 0, 154, 0), ('max', 141, 0, 154, 0), ('min', 142, 0, 146, 0), ('max', 142, 0, 146, 0), ('min', 27, 0, 125, -1), ('max', 27, 0, 125, -1), ('min', 162, 0, 168, 1), ('max', 162, 0, 168, 1), ('min', 40, 0, 158, -1), ('max', 40, 0, 158, -1), ('min', 166, 0, 172, 1), ('max', 166, 0, 172, 1), ('min', 169, 0, 171, 1), ('max', 169, 0, 171, 1), ('min', 170, 0, 173, 0), ('max', 170, 0, 173, 0), ('min', 38, 0, 156, -1), ('max', 38, 0, 156, -1), ('min', 164, 0, 180, 1), ('max', 164, 0, 180, 1), ('min', 42, 0, 160, -1), ('max', 42, 0, 160, -1), ('min', 181, 0, 183, 1), ('max', 181, 0, 183, 1), ('min', 182, 0, 184, 1), ('max', 182, 0, 184, 1), ('min', 175, 0, 179, 1), ('max', 175, 0, 179, 1), ('min', 176, 0, 185, 0), ('max', 176, 0, 185, 0), ('min', 177, 0, 186, 0), ('max', 177, 0, 186, 0), ('min', 178, 0, 187, 0), ('max', 178, 0, 187, 0), ('min', 174, 0, 188, 0), ('max', 174, 0, 188, 0), ('min', 37, 0, 155, -1), ('max', 37, 0, 155, -1), ('min', 163, 0, 200, 1), ('max', 163, 0, 200, 1), ('min', 41, 0, 159, -1), ('max', 41, 0, 159, -1), ('min', 134, 0, 204, 1), ('max', 134, 0, 204, 1), ('min', 201, 0, 203, 1), ('max', 201, 0, 203, 1), ('min', 202, 0, 205, 0), ('max', 202, 0, 205, 0), ('min', 39, 0, 157, -1), ('max', 39, 0, 157, -1), ('min', 165, 0, 212, 1), ('max', 165, 0, 212, 1), ('min', 161, 0, 213, 0), ('max', 161, 0, 213, 0), ('min', 207, 0, 211, 1), ('max', 207, 0, 211, 1), ('min', 208, 0, 215, 0), ('max', 208, 0, 215, 0), ('min', 209, 0, 216, 0), ('max', 209, 0, 216, 0), ('min', 210, 0, 214, 0), ('max', 210, 0, 214, 0), ('min', 189, 0, 199, 1), ('max', 189, 0, 199, 1), ('min', 190, 0, 217, 0), ('max', 190, 0, 217, 0), ('min', 191, 0, 218, 0), ('max', 191, 0, 218, 0), ('min', 192, 0, 219, 0), ('max', 192, 0, 219, 0), ('min', 193, 0, 220, 0), ('max', 193, 0, 220, 0), ('min', 194, 0, 221, 0), ('max', 194, 0, 221, 0), ('min', 195, 0, 222, 0), ('max', 195, 0, 222, 0), ('min', 196, 0, 223, 0), ('max', 196, 0, 223, 0), ('min', 197, 0, 224, 0), ('max', 197, 0, 224, 0), ('min', 198, 0, 206, 0), ('max', 198, 0, 206, 0), ('max', 167, 0, 167, -3), ('min', 240, 0, 240, -3), ('max', 245, 0, 246, -1), ('max', 232, 0, 232, -3), ('min', 247, 0, 248, -1), ('max', 247, 0, 248, -1), ('max', 228, 0, 228, -3), ('min', 244, 0, 244, -3), ('min', 251, 0, 252, 0), ('max', 251, 0, 252, 0), ('min', 236, 0, 236, -3), ('max', 236, 0, 236, -3), ('max', 253, 0, 255, 0), ('min', 254, 0, 256, 0), ('min', 249, 0, 257, -1), ('max', 249, 0, 257, -1), ('min', 250, 0, 258, -1), ('max', 226, 0, 226, -3), ('min', 242, 0, 242, -3), ('min', 262, 0, 263, 0), ('max', 262, 0, 263, 0), ('min', 234, 0, 234, -3), ('max', 234, 0, 234, -3), ('max', 264, 0, 266, 0), ('min', 265, 0, 267, 0), ('max', 230, 0, 230, -3), ('min', 238, 0, 238, -3), ('min', 270, 0, 271, 0), ('max', 270, 0, 271, 0), ('max', 268, 0, 272, 0), ('min', 269, 0, 273, 0), ('max', 269, 0, 273, 0), ('max', 259, 0, 274, -1), ('min', 260, 0, 275, -1), ('max', 260, 0, 275, -1), ('min', 261, 0, 276, -1), ('max', 225, 0, 225, -3), ('min', 241, 0, 241, -3), ('max', 281, 0, 282, 0), ('max', 233, 0, 233, -3), ('min', 283, 0, 284, 0), ('max', 229, 0, 229, -3), ('min', 237, 0, 237, -3), ('max', 286, 0, 287, 0), ('min', 285, 0, 288, 0), ('max', 285, 0, 288, 0), ('max', 227, 0, 227, -3), ('min', 243, 0, 243, -3), ('min', 291, 0, 292, 0), ('max', 291, 0, 292, 0), ('min', 235, 0, 235, -3), ('max', 235, 0, 235, -3), ('max', 293, 0, 295, 0), ('min', 294, 0, 296, 0), ('max', 231, 0, 231, -3), ('min', 239, 0, 239, -3), ('min', 299, 0, 300, 0), ('max', 299, 0, 300, 0), ('max', 297, 0, 301, 0), ('min', 298, 0, 302, 0), ('min', 289, 0, 303, 0), ('max', 289, 0, 303, 0), ('min', 290, 0, 304, 0), ('max', 290, 0, 304, 0), ('min', 277, 0, 305, -1), ('max', 277, 0, 305, -1), ('min', 278, 0, 306, -1), ('max', 278, 0, 306, -1), ('min', 279, 0, 307, -1), ('max', 279, 0, 307, -1), ('min', 280, 0, 308, -1), ('max', 280, 0, 308, -1), ('max', 42, 0, 309, -1), ('max', 41, 0, 310, -1), ('max', 40, 0, 311, -1), ('max', 39, 0, 312, -1), ('max', 38, 0, 313, -1), ('max', 37, 0, 314, -1), ('max', 27, 0, 315, -1), ('min', 317, 0, 318, 0), ('min', 319, 0, 320, 0), ('min', 321, 0, 322, 0), ('min', 316, 0, 323, 1), ('min', 324, 0, 325, 0), ('min', 326, 0, 327, -1), ('min', 328, 0, 329, 0)]
SPANS = [(-3, 3), (-3, 3), (-3, 3), (-3, 3), (-3, 3), (-3, 3), (-3, 3), (-3, 3), (-3, 3), (-3, 3), (-3, 3), (-3, 3), (-3, 3), (-3, 3), (-3, 3), (-3, 3), (-3, 3), (-3, 3), (-3, 3), (-3, 3), (-3, 3), (-3, 3), (-3, 3), (-3, 3), (-3, 3), (-3, 3), (-3, 3), (-3, 3), (-3, 3), (-3, 3), (-3, 3), (-3, 3), (-3, 3), (-3, 3), (-3, 3), (-3, 3), (-3, 3), (-3, 3), (-3, 3), (-3, 3), (-3, 3), (-3, 3), (-3, 3), (0, 0), (0, 0), (0, 0), (0, 0), (0, 0), (0, 0), (0, 0), (1, 1), (1, 1), (1, 1), (1, 1), (1, 1), (-1, 1), (-1, 1), (-1, 1), (-1, 1), (-1, 1), (-1, 1), (-1, 1), (-1, 1), (-1, 1), (-1, 1), (-1, 1), (-1, 1), (-1, 1), (-1, 1), (-1, 1), (-1, 1), (-1, 1), (-1, 1), (-1, 1), (-1, 1), (-1, 1), (-1, 1), (-1, 1), (-1, 1), (-1, 1), (-1, 1), (1, 1), (1, 1), (1, 1), (1, 1), (1, 1), (1, 1), (1, 1), (1, 1), (1, 1), (1, 1), (1, 1), (1, 1), (1, 1), (1, 1), (1, 1), (1, 1), (1, 1), (1, 1), (1, 1), (1, 1), (1, 1), (1, 1), (1, 1), (1, 1), (1, 1), (1, 1), (1, 1), (1, 1), (1, 1), (1, 1), (1, 1), (1, 1), (1, 1), (1, 1), (2, 2), (2, 2), (2, 2), (2, 2), (2, 2), (2, 2), (2, 2), (1, 1), (2, 2), (1, 1), (-2, 1), (-2, 1), (-2, 1), (-2, 1), (-2, 1), (-2, 1), (-2, 1), (-2, 1), (-2, 1), (-2, 1), (-2, 1), (-2, 1), (-2, 1), (-2, 1), (-2, 1), (-2, 1), (-2, 1), (-2, 1), (-2, 1), (-2, 1), (-2, 1), (-2, 1), (-2, 1), (-2, 1), (-2, 1), (-2, 1), (-2, 1), (-2, 1), (-2, 1), (-2, 1), (-2, 1), (-2, 1), (-2, 1), (-2, 1), (-2, 1), (-2, 1), (-2, 1), (-2, 1), (-2, 1), (-2, 1), (-2, 1), (-2, 1), (-1, 2), (-1, 2), (-2, 1), (-2, 1), (-1, 2), (-1, 2), (-2, 1), (-2, 1), (-2, 1), (-2, 1), (-2, 1), (-2, 1), (-1, 2), (-1, 2), (-2, 1), (-2, 1), (-1, 2), (-1, 2), (-2, 1), (-2, 1), (-2, 1), (-2, 1), (-2, 1), (-2, 1), (-2, 1), (-2, 1), (-2, 1), (-2, 1), (-2, 1), (-2, 1), (-2, 1), (-2, 1), (-1, 2), (-1, 2), (-2, 1), (-2, 1), (-1, 2), (-1, 2), (-2, 1), (-2, 1), (-2, 1), (-2, 1), (-2, 1), (-2, 1), (-1, 2), (-1, 2), (-2, 1), (-2, 1), (-2, 1), (-2, 1), (-2, 1), (-2, 1), (-2, 1), (-2, 1), (-2, 1), (-2, 1), (-2, 1), (-2, 1), (-2, 1), (-2, 1), (-2, 1), (-2, 1), (-2, 1), (-2, 1), (-2, 1), (-2, 1), (-2, 1), (-2, 1), (-2, 1), (-2, 1), (-2, 1), (-2, 1), (-2, 1), (-2, 1), (-2, 1), (-2, 1), (-2, 1), (-2, 1), (2, 2), (1, 1), (2, 2), (1, 1), (2, 2), (2, 2), (1, 1), (1, 1), (1, 1), (1, 1), (1, 1), (1, 1), (1, 1), (1, 1), (2, 2), (2, 2), (2, 2), (1, 1), (1, 1), (1, 1), (1, 1), (1, 1), (1, 1), (1, 1), (1, 1), (1, 1), (1, 1), (1, 1), (1, 1), (1, 1), (1, 1), (1, 1), (2, 2), (2, 2), (2, 2), (2, 2), (1, 1), (1, 1), (1, 1), (1, 1), (1, 1), (1, 1), (1, 1), (1, 1), (1, 1), (1, 1), (1, 1), (1, 1), (1, 1), (1, 1), (1, 1), (1, 1), (1, 1), (1, 1), (1, 1), (1, 1), (1, 1), (1, 1), (1, 1), (1, 1), (1, 1), (1, 1), (1, 1), (1, 1), (2, 2), (2, 2), (2, 2), (2, 2), (2, 2), (2, 2), (2, 2), (2, 2), (3, 3), (3, 3), (3, 3), (3, 3), (3, 3), (3, 3), (3, 3), (3, 3), (3, 3), (3, 3), (2, 2), (3, 3), (3, 3), (3, 3)]
OUTS = {'m3': (54, 1), 'm5': (124, 1), 'm7': (330, 3)}
SLOT_OF = [None, None, None, None, None, None, None, 0, 1, 2, 3, 1, 4, 0, 2, 5, 6, 0, 7, 6, 8, 2, 9, 0, 6, 7, 8, 10, 11, 7, 12, 11, 13, 7, 14, 8, 15, 14, 16, 11, 7, 13, 8, 12, 15, 17, 18, 12, 15, 19, 18, 12, 19, 17, None, 3, 15, 4, 1, 12, 18, 15, 4, 19, 17, 12, 15, 18, 4, 20, 21, 22, 23, 20, 4, 19, 18, 17, 22, 23, 12, 21, 15, 3, 21, 18, 12, 21, 15, 3, 22, 21, 4, 18, 22, 20, 15, 12, 22, 20, 17, 22, 20, 19, 23, 3, 20, 19, 17, 23, 22, 21, 20, 19, 1, 18, 17, 23, 22, 21, 20, 19, 18, 17, None, 15, 12, 3, 4, 6, 0, 9, 2, 5, 1, 12, 3, 2, 5, 6, 9, 0, 12, 4, 3, 23, 22, 21, 20, 23, 3, 19, 18, 21, 23, 20, 3, 2, 4, 19, 5, 18, 6, 21, 9, 23, 0, 22, 12, 15, 17, 6, 12, 4, 24, 0, 12, 15, 6, 17, 4, 3, 25, 9, 4, 5, 26, 3, 9, 25, 4, 17, 0, 12, 5, 26, 15, 6, 3, 24, 9, 20, 27, 21, 9, 19, 28, 1, 9, 20, 21, 27, 19, 2, 29, 23, 19, 18, 2, 27, 1, 23, 9, 20, 19, 21, 29, 25, 24, 18, 4, 17, 2, 27, 0, 12, 1, 23, 5, 26, 9, 20, 15, 19, 6, 28, 22, 3, 28, 22, 9, 28, 24, 2, 3, 28, 24, 6, 28, 24, 1, 22, 28, 29, 2, 3, 28, 29, 6, 28, 29, 9, 15, 0, 29, 2, 3, 28, 24, 29, 4, 22, 21, 20, 22, 21, 27, 20, 22, 18, 23, 27, 20, 21, 22, 25, 19, 27, 20, 21, 25, 22, 19, 17, 26, 21, 25, 27, 22, 20, 19, 18, 17, 23, 26, 28, 21, 24, 25, 29, 27, 22, 20, 19, 18, 17, 23, 26, 28, 24, None]
SLOT_WIDTH = {0: 6, 1: 6, 2: 6, 3: 6, 4: 6, 5: 6, 6: 6, 7: 6, 8: 6, 9: 6, 10: 6, 11: 6, 12: 6, 13: 6, 14: 6, 15: 6, 16: 6, 17: 3, 18: 3, 19: 3, 20: 3, 21: 3, 22: 3, 23: 3, 24: 3, 25: 3, 26: 3, 27: 3, 28: 3, 29: 3}

# ------------------------------------------

BF16 = ml_dtypes.bfloat16
W_CHUNK = 128
N_CHUNK = 256 // W_CHUNK
NL = len(LEAVES)
GRP = W_CHUNK // 32           # conv 32-col groups per chunk
N_UNIT = N_CHUNK * GRP
PE_PER_CHUNK = 20             # 12 T-groups + 4 matmul-triples + 4 backT groups
ACT_PER_CHUNK = 20            # 12 fT copies + 4 cT copies + 4 ostage copies
# per-chunk inc orderings (see engine programs):
#  PE:  T3(g0..3)=1..4, T5=5..8, T7=9..12, then mm0=13,mm1=14,bT0=15,mm2=16,
#       bT1=17,mm3=18,bT2=19,bT3=20
#  ACT: fT3(g0..3)=1..4, fT5=5..8, fT7=9..12, then cT(g)=13+2g, ost(g)=14+2g
MM_INC = [13, 14, 16, 18]
BT_INC = [15, 17, 19, 20]

LAST_RESULTS = None
_NC = None

# ---- instruction pairing (fusion) plan ----
# Independent same-op same-width nodes are fused into one DVE instruction
# with a 3D access pattern ([128, 2, w*16]); all value planes live in one
# SBUF slab so a pair is expressible as offset+stride APs.

_OUT_NODES = {v[0] for v in OUTS.values()}
_WIDTH = [W_CHUNK + hi - lo for (lo, hi) in SPANS]


def _slot_key(vi):
    if vi < NL:
        return ('vin', vi)
    if vi in _OUT_NODES:
        return ('m', vi)
    return ('s', SLOT_OF[vi])


def _key_width(key):
    kind, v = key
    if kind == 'vin':
        return _WIDTH[v] * 16
    if kind == 's':
        return (W_CHUNK + SLOT_WIDTH[v]) * 16
    return _WIDTH[v] * 16


def _all_keys():
    return ([('vin', li) for li in range(NL)]
            + [('s', s) for s in SLOT_WIDTH]
            + [('m', vi) for vi in sorted(_OUT_NODES)])


def _layout_slab(order=None):
    """Slab layout (elem col offsets per slot key) in the given key order."""
    off = {}
    cur = 0
    for key in (order or _all_keys()):
        off[key] = cur
        cur += _key_width(key)
    return off, cur


_SLAB_OFF, _SLAB_W = _layout_slab()
_MAX_STEP = 32000  # signed 16-bit ISA stride field


def _node_offsets(n):
    """(out_off, a_off, b_off) for node n."""
    op, ai, ash, bi, bsh = NODES[n]
    vi = NL + n
    lo = SPANS[vi][0]
    o = _SLAB_OFF[_slot_key(vi)]
    a = _SLAB_OFF[_slot_key(ai)] + (lo + ash - SPANS[ai][0]) * 16
    b = _SLAB_OFF[_slot_key(bi)] + (lo + bsh - SPANS[bi][0]) * 16
    return o, a, b


def _key_cap(key):
    """Slot capacity in elems (how many cols may be written from its base)."""
    return _key_width(key)


def _pair_feasible(n0, n1):
    o0, a0, b0 = _node_offsets(n0)
    o1, a1, b1 = _node_offsets(n1)
    if not (abs(o1 - o0) <= _MAX_STEP and abs(a1 - a0) <= _MAX_STEP
            and abs(b1 - b0) <= _MAX_STEP):
        return False
    # cross-width pair runs at the max width: writes must stay inside each
    # out slot; reads must stay inside the slab
    w16 = max(_WIDTH[NL + n0], _WIDTH[NL + n1]) * 16
    if (_key_cap(_slot_key(NL + n0)) < w16
            or _key_cap(_slot_key(NL + n1)) < w16):
        return False
    for off in (a0, b0, a1, b1):
        if off + w16 > _SLAB_W:
            return False
    return True


_LOOKAHEAD = [24]


def _plan_groups(check_strides=True, allow_cross_width=True,
                 wave_fences=True):
    """Greedy pairing of schedule nodes.  Returns list of groups (1-2 node
    indices each) in emission order."""
    bound = _wave_boundaries()
    out_fence = min(v[0] - NL for v in OUTS.values())
    if wave_fences:
        fences = sorted(set(list(bound.values()) + [out_fence]))
    else:
        fences = [out_fence]

    def seg(n):
        s = 0
        for f in fences:
            if n >= f:
                s += 1
        return s

    n_nodes = len(NODES)
    used = [False] * n_nodes
    groups = []
    LOOKAHEAD = _LOOKAHEAD[0]
    for n in range(n_nodes):
        if used[n]:
            continue
        op, a, ash, b, bsh = NODES[n]
        vi = NL + n
        g = [n]
        used[n] = True
        m7_out = OUTS["m7"][0] - NL
        base_out = n in {v[0] - NL for v in OUTS.values()}
        if n != m7_out:
            w0 = _WIDTH[vi]
            gvals = {vi}
            gout = {_slot_key(vi)}
            skip_vals = set()
            skip_written = set()
            skip_read = set()
            m = n + 1
            cand_cross = None
            while m < min(n_nodes, n + 1 + LOOKAHEAD) and len(g) < 4:
                if used[m]:
                    m += 1
                    continue
                op2, a2, ash2, b2, bsh2 = NODES[m]
                vi2 = NL + m
                sk2 = _slot_key(vi2)
                ok = (op2 == op
                      and seg(m) == seg(n)
                      and m != m7_out
                      and not (base_out and vi2 in _OUT_NODES)
                      and a2 not in gvals and b2 not in gvals
                      and sk2 not in gout
                      and _slot_key(a2) not in gout
                      and _slot_key(b2) not in gout
                      and a2 not in skip_vals and b2 not in skip_vals
                      and sk2 not in skip_written
                      and sk2 not in skip_read
                      and (not check_strides or _pair_feasible(n, m)))
                if ok and _WIDTH[vi2] == w0:
                    if len(g) == 1:
                        accept = True
                    else:
                        # extending beyond a pair requires equal deltas
                        o0, a0, b0 = _node_offsets(g[-2])
                        o1, a1, b1 = _node_offsets(g[-1])
                        o2, a2o, b2o = _node_offsets(m)
                        accept = (o2 - o1 == o1 - o0
                                  and a2o - a1 == a1 - a0
                                  and b2o - b1 == b1 - b0
                                  and _WIDTH[NL + g[0]] == w0)
                    if accept:
                        g.append(m)
                        used[m] = True
                        gvals.add(vi2)
                        gout.add(sk2)
                        cand_cross = None
                        m += 1
                        continue
                if ok and allow_cross_width and cand_cross is None and len(g) == 1:
                    cand_cross = m
                skip_vals.add(vi2)
                skip_written.add(sk2)
                skip_read.add(_slot_key(a2))
                skip_read.add(_slot_key(b2))
                m += 1
            if cand_cross is not None and len(g) == 1:
                g.append(cand_cross)
                used[cand_cross] = True
        groups.append(g)
    return groups, bound, fences




# conv plane order = completion order in the schedule (m3 first, m7 last);
# wblk index = weight block column in wpack ([w7|w3|w5|ident])
PLANES = (("m3", 1), ("m5", 2), ("m7", 0))

# leaf-load waves for chunk 0
WAVE = {
    "A1": [li for li, dv in enumerate(LEAVES) if dv in (-1, 0)],
    "A2": [li for li, dv in enumerate(LEAVES) if dv == 1],
    "B": [li for li, dv in enumerate(LEAVES) if dv in (-2, 2)],
    "C": [li for li, dv in enumerate(LEAVES) if dv in (-3, 3)],
}


def _wave_boundaries():
    """First op index needing each wave (None -> op 0)."""
    bound = {}
    for w in ("A2", "B", "C"):
        ws = set(WAVE[w])
        bound[w] = next(n for n, (_o, ai, _as, bi, _bs) in enumerate(NODES)
                        if ai in ws or bi in ws)
    return bound


def _affinity_layout():
    """Order slot keys so that (unconstrained-)paired ops' slots are close,
    maximizing pairs that fit the 16-bit AP stride field."""
    groups0, _, _ = _plan_groups(check_strides=False, allow_cross_width=False)
    import collections
    aff = collections.Counter()
    for g in groups0:
        if len(g) != 2:
            continue
        n0, n1 = g
        _o0, a0, _s, b0, _s2 = NODES[n0]
        _o1, a1, _s3, b1, _s4 = NODES[n1]
        for x, y in (((NL + n0), (NL + n1)), (a0, a1), (b0, b1)):
            kx, ky = _slot_key(x), _slot_key(y)
            if kx != ky:
                aff[frozenset((kx, ky))] += 1
    keys = _all_keys()
    deg = collections.Counter()
    for e, w in aff.items():
        for k in e:
            deg[k] += w
    placed = []
    remaining = set(keys)
    cur = max(remaining, key=lambda k: deg.get(k, 0))
    placed.append(cur)
    remaining.discard(cur)
    while remaining:
        # affinity to the last few placed keys, recency-weighted
        best, bestw = None, -1.0
        for k in remaining:
            w = 0.0
            for d, p in enumerate(reversed(placed[-14:])):
                w += aff.get(frozenset((k, p)), 0) / (1.0 + 0.3 * d)
            if w > bestw:
                best, bestw = k, w
        placed.append(best)
        remaining.discard(best)
    return placed


_SLAB_OFF, _SLAB_W = _layout_slab(_affinity_layout())
def _best_plan(wave_fences):
    best = None
    for la in (16, 24, 32, 40, 56):
        _LOOKAHEAD[0] = la
        for acw in (False,):
            r = _plan_groups(allow_cross_width=acw, wave_fences=wave_fences)
            if best is None or len(r[0]) < len(best[0]):
                best = r
    return best


_GROUPS, _BOUND, _FENCES = _best_plan(True)
_GROUPS1 = _best_plan(False)[0]

# tail-split feasibility: last chunk's final two groups must be exactly
# {m7's two producers} and {m7}, with zero relative column offsets
def _tail_split_ok():
    m7 = OUTS['m7'][0] - NL
    pa, pb = NODES[m7][1] - NL, NODES[m7][3] - NL
    if NODES[m7][2] != 0 or NODES[m7][4] != 0:
        return False, []
    lom = SPANS[NL + m7][0]
    if (SPANS[NL + pa][0] != lom or SPANS[NL + pb][0] != lom
            or _WIDTH[NL + pa] != 128 or _WIDTH[NL + pb] != 128
            or _WIDTH[NL + m7] != 128):
        return False, []
    if len(_GROUPS1) < 2 or _GROUPS1[-1] != [m7]:
        return False, []
    if sorted(_GROUPS1[-2]) != sorted([pa, pb]):
        return False, []
    return True, _GROUPS1[-2]


_TAIL_SPLIT_OK, _TAIL_PAIR = _tail_split_ok()
# emission-order index of the last group touching a leaf (s_cols attach)
_LAST_LEAF_GROUP = max(
    gi for gi, g in enumerate(_GROUPS)
    if any(NODES[n][1] < NL or NODES[n][3] < NL for n in g))


def _build_nc():
    nc = bass.Bass()
    dt = mybir.dt
    x = nc.declare_dram_parameter("x", [134, 262 * 16], dt.bfloat16, False)
    wpack = nc.declare_dram_parameter("wpack", [128, 4 * 128], dt.bfloat16, False)
    outd = nc.declare_dram_parameter("out", [128, 256 * 16], dt.bfloat16, True)

    out_nodes = {v[0] for v in OUTS.values()}
    width = [W_CHUNK + hi - lo for (lo, hi) in SPANS]
    bound = _wave_boundaries()
    last_leaf_op = max(n for n, (_o, ai, _as, bi, _bs) in enumerate(NODES)
                       if ai < NL or bi < NL)
    # op index of each output write, and its conv plane position (0..2)
    out_op = {}   # op index -> (plane_pos, sem_index)
    for pos, (key, _wb) in enumerate(PLANES):
        vi, _osh = OUTS[key]
        out_op[vi - NL] = pos

    ctx = ExitStack()
    _uid = [0]

    def sb(shape, d, name=None):
        _uid[0] += 1
        return ctx.enter_context(
            nc.sbuf_tensor(name or f"t{_uid[0]}", shape, d))

    def ps(shape, d, name=None):
        _uid[0] += 1
        return ctx.enter_context(
            nc.psum_tensor(name or f"p{_uid[0]}", shape, d))

    def sem(name):
        return ctx.enter_context(nc.semaphore(name))

    with ctx:
        wpack_t = sb([128, 512], dt.bfloat16)
        # one slab holds every value plane (leaves, slots, m-planes) so a
        # fused pair of ops is expressible as a 3D offset+stride AP
        slab_off = _SLAB_OFF
        SLAB_W = _SLAB_W
        slab = sb([128, SLAB_W], dt.bfloat16)

        def slab_ap(col, w, r0=0, nr=128):
            """2D AP: [nr partitions, w elems] at column offset col."""
            return slab[r0:r0 + nr, col:col + w]

        _slab_h = slab[:, 0:1].tensor

        def slab_apN(col0, delta, k, w):
            """3D AP: [128, k, w] members at col0 + i*delta."""
            return BassAP(tensor=_slab_h, offset=col0,
                          ap=[[SLAB_W, 128], [delta, k], [1, w]])

        vin_off = [slab_off[('vin', li)] for li in range(NL)]
        m_off = {vi: slab_off[('m', vi)] for vi in out_nodes}
        fT = [[sb([128, 512], dt.bfloat16) for _g in range(GRP)]
              for _p in range(3)]
        cT = [sb([128, 512], dt.bfloat16) for _ in range(2)]
        ostage = [sb([128, 512], dt.bfloat16) for _ in range(2)]
        # plane 2 (m7) is tail-critical: double-buffer it; planes 0/1 single
        psumT = [[ps([128, 512], dt.bfloat16)] for _p in range(2)]
        psumT.append([ps([128, 512], dt.bfloat16) for _g in range(2)])
        psumC = [ps([128, 512], dt.float32) for _ in range(2)]
        psumB = [ps([128, 512], dt.bfloat16) for _ in range(2)]

        s_wave = {w: sem(f"s_{w}") for w in WAVE}
        s_h = sem("s_h")    # A1 col piece [0,60)
        s_h2 = sem("s_h2")  # A1 col piece [60,100)
        s_in1 = sem("s_in1")
        s_cols = sem("s_cols")   # chunk column-stage done (vin consumed)
        s_w = sem("s_w")
        s_m = [sem(f"s_m{i}") for i in range(3)]   # per-plane completion
        s_pe = sem("s_pe")
        s_act = sem("s_act")
        s_out = sem("s_out")

        def leaf_dma(sync, ci, li, row0, nrows, semh):
            lo, hi = SPANS[li]
            dv = LEAVES[li]
            c0 = ci * W_CHUNK + 3 + lo
            sync.dma_start(
                out=slab_ap(vin_off[li], width[li] * 16, row0, nrows),
                in_=x[dv + 3 + row0:dv + 3 + row0 + nrows,
                      c0 * 16:(c0 + width[li]) * 16],
            ).then_inc(semh, 16)

        with nc.Block() as block:

            @block.sync
            def _(sync):
                # wave A1 loads as three column pieces under dedicated
                # semaphores (full counts only — race-free) so the first
                # two (shift-0) ops can start before the full leaf width
                # arrives and the critical last piece is small
                k = 0
                for cs, ce, semh in ((0, 60, s_h), (60, 100, s_h2),
                                     (100, None, s_wave["A1"])):
                    for li in WAVE["A1"]:
                        k += 1
                        if k % 2 == 0:
                            continue  # odd pieces issued by ACT
                        lo, hi = SPANS[li]
                        dv = LEAVES[li]
                        c0 = 3 + lo
                        e = ce if ce is not None else width[li]
                        sync.dma_start(
                            out=slab_ap(vin_off[li] + cs * 16,
                                        (e - cs) * 16),
                            in_=x[dv + 3:dv + 3 + 128,
                                  (c0 + cs) * 16:(c0 + e) * 16],
                        ).then_inc(semh, 16)
                for w, split in (("A2", 4), ("B", 2), ("C", 2)):
                    step = 128 // split
                    for r in range(split):
                        for li in WAVE[w]:
                            k += 1
                            if k % 2 == 0:
                                continue
                            leaf_dma(sync, 0, li, r * step, step, s_wave[w])
                sync.dma_start(out=wpack_t[:], in_=wpack[:]).then_inc(s_w, 16)
                sync.wait_ge(s_cols, 1)
                for li in range(NL):
                    leaf_dma(sync, 1, li, 0, 128, s_in1)
                for u in range(N_UNIT):
                    ci, g = divmod(u, GRP)
                    sync.wait_ge(s_act, ACT_PER_CHUNK * ci + 14 + 2 * g)
                    o0 = (ci * W_CHUNK + g * 32) * 16
                    sync.dma_start(
                        out=outd[0:128, o0:o0 + 512], in_=ostage[u % 2][:]
                    ).then_inc(s_out, 16)

            @block.vector
            def _(vector):
                # value vi lives at slab col base_off(vi); its col 0 is span-lo
                def val_off(vi):
                    return slab_off[_slot_key(vi)]

                def opnd_off(x, lo, sh):
                    """abs slab elem offset of operand value x read at
                    (node-span-lo + shift sh)."""
                    xlo = SPANS[x][0]
                    return val_off(x) + (lo + sh - xlo) * 16


                out_fence = min(out_op)
                m7_op = OUTS["m7"][0] - NL
                for ci in range(N_CHUNK):
                    if ci == 0:
                        for cs, ce, semh in ((0, 60, s_h),
                                             (60, 100, s_h2),
                                             (100, None, s_wave["A1"])):
                            vector.wait_ge(semh, 16 * len(WAVE["A1"]))
                            for g in _GROUPS[:2]:
                                assert len(g) == 1
                                nn0 = g[0]
                                opn, ai, ash, bi, bsh = NODES[nn0]
                                assert ash == 0 and bsh == 0
                                vi = NL + nn0
                                lo = SPANS[vi][0]
                                e = ce if ce is not None else width[vi]
                                vector.tensor_tensor(
                                    slab_ap(val_off(vi) + cs * 16,
                                            (e - cs) * 16),
                                    slab_ap(opnd_off(ai, lo, ash) + cs * 16,
                                            (e - cs) * 16),
                                    slab_ap(opnd_off(bi, lo, bsh) + cs * 16,
                                            (e - cs) * 16),
                                    op=(mybir.AluOpType.min
                                        if opn == "min"
                                        else mybir.AluOpType.max),
                                )
                    else:
                        vector.wait_ge(s_in1, 16 * NL)
                    waved = set()
                    fenced = False
                    glist = _GROUPS if ci == 0 else _GROUPS1
                    for gi, g in enumerate(glist):
                        if ci == 0 and gi < 2:
                            continue
                        n0 = g[0]
                        if ci == 0:
                            for w, split in (("A2", 4), ("B", 2), ("C", 2)):
                                if w not in waved and n0 >= bound[w]:
                                    vector.wait_ge(
                                        s_wave[w],
                                        16 * split * len(WAVE[w]))
                                    waved.add(w)
                        elif not fenced and n0 >= out_fence:
                            # single-buffered m-planes: one conservative wait
                            # (covers all three planes' WAR) at the earliest
                            # output write instead of three 1us bubbles
                            vector.wait_ge(
                                s_pe, PE_PER_CHUNK * (ci - 1) + 12)
                            fenced = True
                        op = NODES[n0][0]
                        alu = (mybir.AluOpType.min if op == "min"
                               else mybir.AluOpType.max)
                        # last chunk: interleave column-halves of the final
                        # m7 producers with the m7 quarters so the conv tail
                        # starts ~2 ops earlier (all three have span-lo 3 and
                        # shift 0, so columns align exactly)
                        if (ci == N_CHUNK - 1 and gi == len(glist) - 2
                                and _TAIL_SPLIT_OK
                                and sorted(g) == sorted(_TAIL_PAIR)):
                            p0, p1 = g
                            oo, aa, bb = [], [], []
                            for n_ in g:
                                _o, a_, as_, b_, bs_ = NODES[n_]
                                v_ = NL + n_
                                lo_ = SPANS[v_][0]
                                oo.append(val_off(v_))
                                aa.append(opnd_off(a_, lo_, as_))
                                bb.append(opnd_off(b_, lo_, bs_))
                            opm, am, asm, bm, bsm = NODES[m7_op]
                            vm = NL + m7_op
                            lom = SPANS[vm][0]
                            alum = (mybir.AluOpType.min if opm == "min"
                                    else mybir.AluOpType.max)
                            for h in range(2):
                                ho = h * 64 * 16
                                vector.tensor_tensor(
                                    slab_apN(oo[0] + ho, oo[1] - oo[0],
                                             2, 64 * 16),
                                    slab_apN(aa[0] + ho, aa[1] - aa[0],
                                             2, 64 * 16),
                                    slab_apN(bb[0] + ho, bb[1] - bb[0],
                                             2, 64 * 16),
                                    op=alu,
                                )
                                for q in (2 * h, 2 * h + 1):
                                    qo = q * 32 * 16
                                    vector.tensor_tensor(
                                        slab_ap(val_off(vm) + qo, 32 * 16),
                                        slab_ap(opnd_off(am, lom, asm) + qo,
                                                32 * 16),
                                        slab_ap(opnd_off(bm, lom, bsm) + qo,
                                                32 * 16),
                                        op=alum,
                                    ).then_inc(s_m[2], 1)
                            continue
                        if (ci == N_CHUNK - 1 and gi == len(glist) - 1
                                and _TAIL_SPLIT_OK):
                            continue  # m7 already emitted interleaved above
                        if len(g) == 1 and n0 == m7_op:
                            # final m7 op: emit in 4 col-quarters, inc
                            # s_m[2] per quarter so PE transposes of group
                            # g start before the full plane is done
                            op_, ai, ash, bi, bsh = NODES[n0]
                            vi = NL + n0
                            lo = SPANS[vi][0]
                            for q in range(4):
                                qo = q * 32 * 16
                                vector.tensor_tensor(
                                    slab_ap(val_off(vi) + qo, 32 * 16),
                                    slab_ap(opnd_off(ai, lo, ash) + qo,
                                            32 * 16),
                                    slab_ap(opnd_off(bi, lo, bsh) + qo,
                                            32 * 16),
                                    op=alu,
                                ).then_inc(s_m[2], 1)
                            continue
                        if len(g) == 1:
                            op_, ai, ash, bi, bsh = NODES[n0]
                            vi = NL + n0
                            lo = SPANS[vi][0]
                            wdt = width[vi]
                            inst = vector.tensor_tensor(
                                slab_ap(val_off(vi), wdt * 16),
                                slab_ap(opnd_off(ai, lo, ash), wdt * 16),
                                slab_ap(opnd_off(bi, lo, bsh), wdt * 16),
                                op=alu,
                            )
                        else:
                            oo, aa, bb = [], [], []
                            for n_ in g:
                                _o, a_, as_, b_, bs_ = NODES[n_]
                                v_ = NL + n_
                                lo_ = SPANS[v_][0]
                                oo.append(val_off(v_))
                                aa.append(opnd_off(a_, lo_, as_))
                                bb.append(opnd_off(b_, lo_, bs_))
                            w16 = max(width[NL + n_] for n_ in g) * 16
                            k = len(g)
                            inst = vector.tensor_tensor(
                                slab_apN(oo[0], oo[1] - oo[0], k, w16),
                                slab_apN(aa[0], aa[1] - aa[0], k, w16),
                                slab_apN(bb[0], bb[1] - bb[0], k, w16),
                                op=alu,
                            )
                        for n in g:
                            if n in out_op:
                                inst.then_inc(s_m[out_op[n]], 1)
                        if ci == 0 and gi == _LAST_LEAF_GROUP:
                            inst.then_inc(s_cols, 1)
                    if ci == N_CHUNK - 1:
                        # DVE is idle after its last op: take over the tail
                        # cT/ostage copies from ACT to shorten the epilogue
                        pb = PE_PER_CHUNK * ci
                        for g in range(GRP):
                            u = ci * GRP + g
                            # cT double-buffered: WAR is bT(g-2), and
                            # max(MM_INC[g], BT_INC[g-2]) == MM_INC[g]
                            vector.wait_ge(s_pe, pb + MM_INC[g])
                            vector.tensor_copy(
                                cT[g % 2][:],
                                psumC[g % 2][:]).then_inc(s_act, 1)
                            vector.wait_ge(s_pe, pb + BT_INC[g])
                            if u >= 2:
                                vector.wait_ge(s_out, 16 * (u - 1))
                            vector.tensor_copy(
                                ostage[u % 2][:],
                                psumB[g % 2][:]).then_inc(s_act, 1)

            @block.tensor
            def _(tensor):
                tensor.wait_ge(s_w, 16)
                for ci in range(N_CHUNK):
                    pb = PE_PER_CHUNK * ci
                    ab = ACT_PER_CHUNK * ci
                    # transpose phases, plane by plane (completion order)
                    for p, (key, _wb) in enumerate(PLANES):
                        if p == 2 and ci == N_CHUNK - 1:
                            mm_a(0)
                            mm_a(1)
                        vi, osh = OUTS[key]
                        moff = m_off[vi]
                        start = osh - SPANS[vi][0]
                        if p != 2:
                            tensor.wait_ge(s_m[p], ci + 1)
                        nb = len(psumT[p])
                        for g in range(GRP):
                            if p == 2:
                                # m7 arrives in col-quarters (4 incs/chunk)
                                tensor.wait_ge(s_m[2], 4 * ci + g + 1)
                            # psumT[p][g%nb] WAR: fT_p copy nb groups back
                            war = ab + 4 * p + g + 1 - nb
                            if war >= 1:
                                tensor.wait_ge(s_act, war)
                            for blk in range(4):
                                col0 = (start + g * 32 + blk * 8) * 16
                                inst = nc.tensor.transpose(
                                    psumT[p][g % nb][:, blk * 128:(blk + 1) * 128],
                                    slab_ap(moff + col0, 128),
                                    wpack_t[:, 384:512],
                                )
                            inst.then_inc(s_pe, 1)
                    # last chunk: m3+m5 products for groups 0/1 issue
                    # mid-chunk (before the m7-gated T7 transposes), leaving
                    # only the m7 matmul on the tail critical path
                    def mm_a(g):
                        # fT5(g) inc (ab+5+g) implies fT3(g) (ab+1+g)
                        tensor.wait_ge(s_act, ab + 5 + g)
                        if g >= 2:
                            tensor.wait_ge(s_act, ab + 9 + 2 * g)
                        for p, (_key, wb) in enumerate(PLANES[:2]):
                            nc.tensor.matmul(
                                psumC[g % 2][:],
                                wpack_t[:, wb * 128:(wb + 1) * 128],
                                fT[p][g][:], start=(p == 0), stop=False,
                            )

                    def mm_b(g):
                        tensor.wait_ge(s_act, ab + 9 + g)
                        _key, wb = PLANES[2]
                        nc.tensor.matmul(
                            psumC[g % 2][:],
                            wpack_t[:, wb * 128:(wb + 1) * 128],
                            fT[2][g][:], start=False, stop=True,
                        ).then_inc(s_pe, 1)

                    # matmul triples and back-transposes, interleaved
                    def mm(g):
                        tensor.wait_ge(s_act, ab + 9 + g)
                        if g >= 2:
                            tensor.wait_ge(s_act, ab + 9 + 2 * g)
                        for p, (_key, wb) in enumerate(PLANES):
                            inst = nc.tensor.matmul(
                                psumC[g % 2][:],
                                wpack_t[:, wb * 128:(wb + 1) * 128],
                                fT[p][g][:], start=(p == 0), stop=(p == 2),
                            )
                        inst.then_inc(s_pe, 1)

                    def bT(g):
                        tensor.wait_ge(s_act, ab + 13 + 2 * g)
                        for blk in range(4):
                            inst = nc.tensor.transpose(
                                psumB[g % 2][:, blk * 128:(blk + 1) * 128],
                                cT[g % 2][:, blk * 128:(blk + 1) * 128],
                                wpack_t[:, 384:512],
                            )
                        inst.then_inc(s_pe, 1)

                    if ci == N_CHUNK - 1:
                        mm_b(0); mm_b(1); bT(0); mm(2); bT(1); mm(3)
                        bT(2); bT(3)
                    else:
                        mm(0); mm(1); bT(0); mm(2); bT(1); mm(3)
                        bT(2); bT(3)

            @block.scalar
            def _(scalar):
                k = 0
                for cs, ce, semh in ((0, 60, s_h), (60, 100, s_h2),
                                     (100, None, s_wave["A1"])):
                    for li in WAVE["A1"]:
                        k += 1
                        if k % 2 == 1:
                            continue  # even pieces issued by SP
                        lo, hi = SPANS[li]
                        dv = LEAVES[li]
                        c0 = 3 + lo
                        e = ce if ce is not None else width[li]
                        scalar.dma_start(
                            out=slab_ap(vin_off[li] + cs * 16,
                                        (e - cs) * 16),
                            in_=x[dv + 3:dv + 3 + 128,
                                  (c0 + cs) * 16:(c0 + e) * 16],
                        ).then_inc(semh, 16)
                for w, split in (("A2", 4), ("B", 2), ("C", 2)):
                    step = 128 // split
                    for r in range(split):
                        for li in WAVE[w]:
                            k += 1
                            if k % 2 == 1:
                                continue
                            leaf_dma(scalar, 0, li, r * step, step,
                                     s_wave[w])
                for ci in range(N_CHUNK):
                    pb = PE_PER_CHUNK * ci
                    for p in range(3):
                        nb = len(psumT[p])
                        for g in range(GRP):
                            scalar.wait_ge(s_pe, pb + 4 * p + g + 1)
                            scalar.copy(fT[p][g][:],
                                        psumT[p][g % nb][:]).then_inc(s_act, 1)
                    if ci == N_CHUNK - 1:
                        continue  # tail copies run on DVE (idle by then)
                    for g in range(GRP):
                        u = ci * GRP + g
                        # cT double-buffered: WAR is bT(g-2), and
                        # max(MM_INC[g], BT_INC[g-2]) == MM_INC[g]
                        scalar.wait_ge(s_pe, pb + MM_INC[g])
                        scalar.copy(cT[g % 2][:],
                                    psumC[g % 2][:]).then_inc(s_act, 1)
                        scalar.wait_ge(s_pe, pb + BT_INC[g])
                        if u >= 2:
                            scalar.wait_ge(s_out, 16 * (u - 1))
                        scalar.copy(ostage[u % 2][:],
                                    psumB[g % 2][:]).then_inc(s_act, 1)

    return nc


def _get_nc():
    global _NC
    if _NC is None:
        _NC = _build_nc()
    return _NC


def kernel(x, high_kernel):
    global LAST_RESULTS
    if not os.environ.get("TRN_TRACE"):
        # tracing needs an NTFF hook this image may not have; make sure a
        # stray BASS_TRACE in the environment can't activate it
        os.environ.setdefault("BASS_NEVER_TRACE", "1")
    x = np.asarray(x, dtype=np.float32)
    hk = np.asarray(high_kernel, dtype=np.float32)
    B, H, W, C = x.shape
    assert (B, H, W, C) == (4, 256, 256, 16)

    xp = np.zeros((B, H + 6, W + 6, C), np.float32)
    xp[:, 3:-3, 3:-3, :] = x
    xbf = xp.astype(BF16)

    wm = hk[0, 0]  # [48, 16] (c, o)
    wpack = np.zeros((128, 512), np.float32)
    for p, sl in enumerate((slice(0, 16), slice(16, 32), slice(32, 48))):
        sub = wm[sl]
        for wl in range(8):
            wpack[wl * 16:(wl + 1) * 16,
                  p * 128 + wl * 16:p * 128 + (wl + 1) * 16] = sub
    wpack[:, 384:512] = np.eye(128)
    wpack = wpack.astype(BF16)

    in_maps = []
    for core in range(8):
        b, hh = divmod(core, 2)
        sh = np.ascontiguousarray(
            xbf[b, hh * 128: hh * 128 + 134].reshape(134, 262 * 16))
        in_maps.append({"x": sh, "wpack": wpack})

    nc = _get_nc()
    res = run_bass_kernel_spmd(
        nc, in_maps, core_ids=list(range(8)),
        trace=bool(os.environ.get("TRN_TRACE")),
    )
    LAST_RESULTS = res

    outf = np.empty((4, 256, 256, 16), np.float32)
    for core in range(8):
        b, hh = divmod(core, 2)
        o = np.asarray(res.results[core]["out"]).astype(np.float32)
        outf[b, hh * 128:(hh + 1) * 128] = o.reshape(128, 256, 16)
    return outf



# revision 55
# speedup vs baseline: 1.1978x; 1.1978x over previous
"""Trainium2 Bass kernel for nn_AdaptiveMBlock (three rank-select pools + 1x1 conv).

Self-contained: the selection-network schedule is baked in below.

Strategy:
  - 8 cores = (batch 4) x (H halves of 128 rows).  Host pads H and W by 3
    with zeros (SAME padding), casts to bf16, ships each core a
    [134, 262*16] shard.
  - On-chip layout: partitions = 128 H rows, free dim = (W, C) with C
    contiguous.  Vertical window offsets come from 7 row-shifted DMA loads;
    horizontal offsets are free-dim access-pattern offsets (free).
  - Column stage: shared sorting networks produce sorted 3/5/7-tall column
    planes; horizontal stage: hash-consed pruned odd-even merge networks
    select rank 6/9, 14/25, 26/49 per pixel.  All ops are DVE
    tensor_tensor min/max in bf16 (2x perf mode).
  - 1x1 conv (48->16) on the TensorEngine via per-128-block transposes and
    block-diagonal weights; bf16 result written back transposed into PSUM
    and DMAed straight from PSUM; host casts to f32.
  - Raw bass (no Tile): explicit per-engine programs with counting
    semaphores, each instruction carrying at most one wait (this
    container's walrus rejects multi-wait instructions).
  - Two 128-wide W chunks.  Chunk-0 leaf loads are staged in four waves
    split across DMA queues so DVE starts early.  m-planes complete in
    order m3 < m5 < m7; the conv consumes planes in that order via three
    per-plane semaphores so transposes/copies overlap the tail.
  - All value planes (leaves, slots, m-planes) live in ONE SBUF slab;
    independent same-op same-width schedule nodes are fused pairwise into
    single DVE instructions via 3D offset+stride APs (~40% fewer DVE
    instructions; saves the ~150ns per-instruction overhead).  The slab
    slot order is an offline local-search result maximizing pairs whose
    AP strides fit the signed-16-bit ISA field.
  - The final m7 op is emitted in four 32-col quarters incrementing the
    plane semaphore each, so the conv tail starts ~4 transposes earlier.
    (NOTE: partial-count waits on DMA wave semaphores are UNSAFE — piece
    completions across the 16 DMA engines are unordered; only full-wave
    counts may be waited on.)
"""
import os
import sys
from contextlib import ExitStack

sys.path.insert(0, "/opt/trn_rl_repo")

import numpy as np
import ml_dtypes

import concourse.bass as bass
import concourse.mybir as mybir
from concourse.ap import AP as BassAP
from concourse.bass_utils import run_bass_kernel_spmd

# ---- baked selection-network schedule ----
# Auto-generated by bake_schedule.py — selection network schedule
LEAVES = [-3, -2, -1, 0, 1, 2, 3]
NODES = [('min', 2, 0, 3, 0), ('max', 2, 0, 3, 0), ('min', 4, 0, 8, 0), ('max', 4, 0, 8, 0), ('min', 7, 0, 9, 0), ('max', 7, 0, 9, 0), ('min', 1, 0, 5, 0), ('max', 1, 0, 5, 0), ('min', 11, 0, 13, 0), ('max', 11, 0, 13, 0), ('min', 10, 0, 16, 0), ('max', 10, 0, 16, 0), ('min', 12, 0, 14, 0), ('max', 12, 0, 14, 0), ('min', 17, 0, 19, 0), ('max', 17, 0, 19, 0), ('min', 18, 0, 20, 0), ('max', 18, 0, 20, 0), ('min', 0, 0, 6, 0), ('max', 0, 0, 6, 0), ('min', 15, 0, 25, 0), ('max', 15, 0, 25, 0), ('min', 24, 0, 28, 0), ('max', 24, 0, 28, 0), ('min', 22, 0, 29, 0), ('max', 22, 0, 29, 0), ('min', 21, 0, 26, 0), ('max', 21, 0, 26, 0), ('min', 23, 0, 34, 0), ('max', 23, 0, 34, 0), ('min', 31, 0, 33, 0), ('max', 31, 0, 33, 0), ('min', 32, 0, 35, 0), ('max', 32, 0, 35, 0), ('min', 30, 0, 36, 0), ('max', 30, 0, 36, 0), ('max', 11, 0, 11, -1), ('min', 10, 0, 10, -1), ('max', 10, 0, 10, -1), ('max', 43, 0, 44, 0), ('max', 12, 0, 12, -1), ('min', 46, 0, 47, 0), ('max', 46, 0, 47, 0), ('max', 10, 0, 48, -1), ('max', 12, 0, 49, -1), ('max', 11, 0, 45, -1), ('min', 50, 0, 51, 0), ('min', 52, 0, 53, 0), ('min', 15, 0, 15, -1), ('max', 15, 0, 15, -1), ('min', 24, 0, 24, -1), ('max', 24, 0, 24, -1), ('min', 56, 0, 57, 0), ('max', 56, 0, 57, 0), ('min', 22, 0, 22, -1), ('max', 22, 0, 22, -1), ('min', 59, 0, 61, 0), ('max', 59, 0, 61, 0), ('min', 60, 0, 62, 0), ('max', 60, 0, 62, 0), ('min', 21, 0, 21, -1), ('max', 21, 0, 21, -1), ('min', 23, 0, 23, -1), ('max', 23, 0, 23, -1), ('min', 68, 0, 69, 0), ('max', 68, 0, 69, 0), ('min', 63, 0, 67, 0), ('max', 63, 0, 67, 0), ('min', 64, 0, 71, 0), ('max', 64, 0, 71, 0), ('min', 65, 0, 72, 0), ('max', 65, 0, 72, 0), ('min', 66, 0, 70, 0), ('max', 66, 0, 70, 0), ('max', 55, 0, 55, -2), ('min', 80, 0, 80, -2), ('max', 81, 0, 82, 0), ('max', 76, 0, 76, -2), ('min', 83, 0, 84, 0), ('max', 83, 0, 84, 0), ('max', 74, 0, 74, -2), ('min', 78, 0, 78, -2), ('max', 78, 0, 78, -2), ('max', 87, 0, 88, 0), ('min', 85, 0, 90, 0), ('max', 85, 0, 90, 0), ('min', 86, 0, 89, 0), ('max', 73, 0, 73, -2), ('min', 58, 0, 58, -2), ('min', 94, 0, 95, 0), ('max', 94, 0, 95, 0), ('min', 77, 0, 77, -2), ('max', 77, 0, 77, -2), ('max', 96, 0, 98, 0), ('min', 97, 0, 99, 0), ('max', 75, 0, 75, -2), ('min', 79, 0, 79, -2), ('min', 102, 0, 103, 0), ('max', 102, 0, 103, 0), ('max', 100, 0, 104, 0), ('min', 101, 0, 105, 0), ('max', 101, 0, 105, 0), ('min', 91, 0, 106, 0), ('max', 91, 0, 106, 0), ('min', 92, 0, 107, 0), ('max', 92, 0, 107, 0), ('min', 93, 0, 108, 0), ('max', 93, 0, 108, 0), ('max', 24, 0, 109, -1), ('max', 23, 0, 110, -1), ('max', 22, 0, 111, -1), ('max', 21, 0, 112, -1), ('max', 15, 0, 113, -1), ('min', 115, 0, 116, 0), ('min', 117, 0, 118, 0), ('min', 114, 0, 119, 1), ('min', 120, 0, 121, 0), ('min', 122, 0, 123, 1), ('min', 27, 0, 27, -1), ('max', 27, 0, 27, -1), ('min', 40, 0, 40, -1), ('max', 40, 0, 40, -1), ('min', 126, 0, 127, 0), ('max', 126, 0, 127, 0), ('min', 38, 0, 38, -1), ('max', 38, 0, 38, -1), ('min', 42, 0, 42, -1), ('max', 42, 0, 42, -1), ('min', 132, 0, 133, 0), ('max', 132, 0, 133, 0), ('min', 129, 0, 131, 0), ('max', 129, 0, 131, 0), ('min', 130, 0, 135, 0), ('max', 130, 0, 135, 0), ('min', 128, 0, 136, 0), ('max', 128, 0, 136, 0), ('min', 37, 0, 37, -1), ('max', 37, 0, 37, -1), ('min', 41, 0, 41, -1), ('max', 41, 0, 41, -1), ('min', 144, 0, 145, 0), ('max', 144, 0, 145, 0), ('min', 39, 0, 39, -1), ('max', 39, 0, 39, -1), ('min', 147, 0, 149, 0), ('max', 147, 0, 149, 0), ('min', 148, 0, 150, 0), ('max', 148, 0, 150, 0), ('min', 137, 0, 143, 0), ('max', 137, 0, 143, 0), ('min', 138, 0, 151, 0), ('max', 138, 0, 151, 0), ('min', 139, 0, 152, 0), ('max', 139, 0, 152, 0), ('min', 140, 0, 153, 0), ('max', 140, 0, 153, 0), ('min', 141, 0, 154, 0), ('max', 141, 0, 154, 0), ('min', 142, 0, 146, 0), ('max', 142, 0, 146, 0), ('min', 27, 0, 125, -1), ('max', 27, 0, 125, -1), ('min', 162, 0, 168, 1), ('max', 162, 0, 168, 1), ('min', 40, 0, 158, -1), ('max', 40, 0, 158, -1), ('min', 166, 0, 172, 1), ('max', 166, 0, 172, 1), ('min', 169, 0, 171, 1), ('max', 169, 0, 171, 1), ('min', 170, 0, 173, 0), ('max', 170, 0, 173, 0), ('min', 38, 0, 156, -1), ('max', 38, 0, 156, -1), ('min', 164, 0, 180, 1), ('max', 164, 0, 180, 1), ('min', 42, 0, 160, -1), ('max', 42, 0, 160, -1), ('min', 181, 0, 183, 1), ('max', 181, 0, 183, 1), ('min', 182, 0, 184, 1), ('max', 182, 0, 184, 1), ('min', 175, 0, 179, 1), ('max', 175, 0, 179, 1), ('min', 176, 0, 185, 0), ('max', 176, 0, 185, 0), ('min', 177, 0, 186, 0), ('max', 177, 0, 186, 0), ('min', 178, 0, 187, 0), ('max', 178, 0, 187, 0), ('min', 174, 0, 188, 0), ('max', 174, 0, 188, 0), ('min', 37, 0, 155, -1), ('max', 37, 0, 155, -1), ('min', 163, 0, 200, 1), ('max', 163, 0, 200, 1), ('min', 41, 0, 159, -1), ('max', 41, 0, 159, -1), ('min', 134, 0, 204, 1), ('max', 134, 0, 204, 1), ('min', 201, 0, 203, 1), ('max', 201, 0, 203, 1), ('min', 202, 0, 205, 0), ('max', 202, 0, 205, 0), ('min', 39, 0, 157, -1), ('max', 39, 0, 157, -1), ('min', 165, 0, 212, 1), ('max', 165, 0, 212, 1), ('min', 161, 0, 213, 0), ('max', 161, 0, 213, 0), ('min', 207, 0, 211, 1), ('max', 207, 0, 211, 1), ('min', 208, 0, 215, 0), ('max', 208, 0, 215, 0), ('min', 209, 0, 216, 0), ('max', 209, 0, 216, 0), ('min', 210, 0, 214, 0), ('max', 210, 0, 214, 0), ('min', 189, 0, 199, 1), ('max', 189, 0, 199, 1), ('min', 190, 0, 217, 0), ('max', 190, 0, 217, 0), ('min', 191, 0, 218, 0), ('max', 191, 0, 218, 0), ('min', 192, 0, 219, 0), ('max', 192, 0, 219, 0), ('min', 193, 0, 220, 0), ('max', 193, 0, 220, 0), ('min', 194, 0, 221, 0), ('max', 194, 0, 221, 0), ('min', 195, 0, 222, 0), ('max', 195, 0, 222, 0), ('min', 196, 0, 223, 0), ('max', 196, 0, 223, 0), ('min', 197, 0, 224, 0), ('max', 197, 0, 224, 0), ('min', 198, 0, 206, 0), ('max', 198, 0, 206, 0), ('max', 167, 0, 167, -3), ('min', 240, 0, 240, -3), ('max', 245, 0, 246, -1), ('max', 232, 0, 232, -3), ('min', 247, 0, 248, -1), ('max', 247, 0, 248, -1), ('max', 228, 0, 228, -3), ('min', 244, 0, 244, -3), ('min', 251, 0, 252, 0), ('max', 251, 0, 252, 0), ('min', 236, 0, 236, -3), ('max', 236, 0, 236, -3), ('max', 253, 0, 255, 0), ('min', 254, 0, 256, 0), ('min', 249, 0, 257, -1), ('max', 249, 0, 257, -1), ('min', 250, 0, 258, -1), ('max', 226, 0, 226, -3), ('min', 242, 0, 242, -3), ('min', 262, 0, 263, 0), ('max', 262, 0, 263, 0), ('min', 234, 0, 234, -3), ('max', 234, 0, 234, -3), ('max', 264, 0, 266, 0), ('min', 265, 0, 267, 0), ('max', 230, 0, 230, -3), ('min', 238, 0, 238, -3), ('min', 270, 0, 271, 0), ('max', 270, 0, 271, 0), ('max', 268, 0, 272, 0), ('min', 269, 0, 273, 0), ('max', 269, 0, 273, 0), ('max', 259, 0, 274, -1), ('min', 260, 0, 275, -1), ('max', 260, 0, 275, -1), ('min', 261, 0, 276, -1), ('max', 225, 0, 225, -3), ('min', 241, 0, 241, -3), ('max', 281, 0, 282, 0), ('max', 233, 0, 233, -3), ('min', 283, 0, 284, 0), ('max', 229, 0, 229, -3), ('min', 237, 0, 237, -3), ('max', 286, 0, 287, 0), ('min', 285, 0, 288, 0), ('max', 285, 0, 288, 0), ('max', 227, 0, 227, -3), ('min', 243, 0, 243, -3), ('min', 291, 0, 292, 0), ('max', 291, 0, 292, 0), ('min', 235, 0, 235, -3), ('max', 235, 0, 235, -3), ('max', 293, 0, 295, 0), ('min', 294, 0, 296, 0), ('max', 231, 0, 231, -3), ('min', 239, 0, 239, -3), ('min', 299, 0, 300, 0), ('max', 299, 0, 300, 0), ('max', 297, 0, 301, 0), ('min', 298, 0, 302, 0), ('min', 289, 0, 303, 0), ('max', 289, 0, 303, 0), ('min', 290, 0, 304, 0), ('max', 290, 0, 304, 0), ('min', 277, 0, 305, -1), ('max', 277, 0, 305, -1), ('min', 278, 0, 306, -1), ('max', 278, 0, 306, -1), ('min', 279, 0, 307, -1), ('max', 279, 0, 307, -1), ('min', 280, 0, 308, -1), ('max', 280, 0, 308, -1), ('max', 42, 0, 309, -1), ('max', 41, 0, 310, -1), ('max', 40, 0, 311, -1), ('max', 39, 0, 312, -1), ('max', 38, 0, 313, -1), ('max', 37, 0, 314, -1), ('max', 27, 0, 315, -1), ('min', 317, 0, 318, 0), ('min', 319, 0, 320, 0), ('min', 321, 0, 322, 0), ('min', 316, 0, 323, 1), ('min', 324, 0, 325, 0), ('min', 326, 0, 327, -1), ('min', 328, 0, 329, 0)]
SPANS = [(-3, 3), (-3, 3), (-3, 3), (-3, 3), (-3, 3), (-3, 3), (-3, 3), (-3, 3), (-3, 3), (-3, 3), (-3, 3), (-3, 3), (-3, 3), (-3, 3), (-3, 3), (-3, 3), (-3, 3), (-3, 3), (-3, 3), (-3, 3), (-3, 3), (-3, 3), (-3, 3), (-3, 3), (-3, 3), (-3, 3), (-3, 3), (-3, 3), (-3, 3), (-3, 3), (-3, 3), (-3, 3), (-3, 3), (-3, 3), (-3, 3), (-3, 3), (-3, 3), (-3, 3), (-3, 3), (-3, 3), (-3, 3), (-3, 3), (-3, 3), (0, 0), (0, 0), (0, 0), (0, 0), (0, 0), (0, 0), (0, 0), (1, 1), (1, 1), (1, 1), (1, 1), (1, 1), (-1, 1), (-1, 1), (-1, 1), (-1, 1), (-1, 1), (-1, 1), (-1, 1), (-1, 1), (-1, 1), (-1, 1), (-1, 1), (-1, 1), (-1, 1), (-1, 1), (-1, 1), (-1, 1), (-1, 1), (-1, 1), (-1, 1), (-1, 1), (-1, 1), (-1, 1), (-1, 1), (-1, 1), (-1, 1), (-1, 1), (1, 1), (1, 1), (1, 1), (1, 1), (1, 1), (1, 1), (1, 1), (1, 1), (1, 1), (1, 1), (1, 1), (1, 1), (1, 1), (1, 1), (1, 1), (1, 1), (1, 1), (1, 1), (1, 1), (1, 1), (1, 1), (1, 1), (1, 1), (1, 1), (1, 1), (1, 1), (1, 1), (1, 1), (1, 1), (1, 1), (1, 1), (1, 1), (1, 1), (1, 1), (2, 2), (2, 2), (2, 2), (2, 2), (2, 2), (2, 2), (2, 2), (1, 1), (2, 2), (1, 1), (-2, 1), (-2, 1), (-2, 1), (-2, 1), (-2, 1), (-2, 1), (-2, 1), (-2, 1), (-2, 1), (-2, 1), (-2, 1), (-2, 1), (-2, 1), (-2, 1), (-2, 1), (-2, 1), (-2, 1), (-2, 1), (-2, 1), (-2, 1), (-2, 1), (-2, 1), (-2, 1), (-2, 1), (-2, 1), (-2, 1), (-2, 1), (-2, 1), (-2, 1), (-2, 1), (-2, 1), (-2, 1), (-2, 1), (-2, 1), (-2, 1), (-2, 1), (-2, 1), (-2, 1), (-2, 1), (-2, 1), (-2, 1), (-2, 1), (-1, 2), (-1, 2), (-2, 1), (-2, 1), (-1, 2), (-1, 2), (-2, 1), (-2, 1), (-2, 1), (-2, 1), (-2, 1), (-2, 1), (-1, 2), (-1, 2), (-2, 1), (-2, 1), (-1, 2), (-1, 2), (-2, 1), (-2, 1), (-2, 1), (-2, 1), (-2, 1), (-2, 1), (-2, 1), (-2, 1), (-2, 1), (-2, 1), (-2, 1), (-2, 1), (-2, 1), (-2, 1), (-1, 2), (-1, 2), (-2, 1), (-2, 1), (-1, 2), (-1, 2), (-2, 1), (-2, 1), (-2, 1), (-2, 1), (-2, 1), (-2, 1), (-1, 2), (-1, 2), (-2, 1), (-2, 1), (-2, 1), (-2, 1), (-2, 1), (-2, 1), (-2, 1), (-2, 1), (-2, 1), (-2, 1), (-2, 1), (-2, 1), (-2, 1), (-2, 1), (-2, 1), (-2, 1), (-2, 1), (-2, 1), (-2, 1), (-2, 1), (-2, 1), (-2, 1), (-2, 1), (-2, 1), (-2, 1), (-2, 1), (-2, 1), (-2, 1), (-2, 1), (-2, 1), (-2, 1), (-2, 1), (2, 2), (1, 1), (2, 2), (1, 1), (2, 2), (2, 2), (1, 1), (1, 1), (1, 1), (1, 1), (1, 1), (1, 1), (1, 1), (1, 1), (2, 2), (2, 2), (2, 2), (1, 1), (1, 1), (1, 1), (1, 1), (1, 1), (1, 1), (1, 1), (1, 1), (1, 1), (1, 1), (1, 1), (1, 1), (1, 1), (1, 1), (1, 1), (2, 2), (2, 2), (2, 2), (2, 2), (1, 1), (1, 1), (1, 1), (1, 1), (1, 1), (1, 1), (1, 1), (1, 1), (1, 1), (1, 1), (1, 1), (1, 1), (1, 1), (1, 1), (1, 1), (1, 1), (1, 1), (1, 1), (1, 1), (1, 1), (1, 1), (1, 1), (1, 1), (1, 1), (1, 1), (1, 1), (1, 1), (1, 1), (2, 2), (2, 2), (2, 2), (2, 2), (2, 2), (2, 2), (2, 2), (2, 2), (3, 3), (3, 3), (3, 3), (3, 3), (3, 3), (3, 3), (3, 3), (3, 3), (3, 3), (3, 3), (2, 2), (3, 3), (3, 3), (3, 3)]
OUTS = {'m3': (54, 1), 'm5': (124, 1), 'm7': (330, 3)}
SLOT_OF = [None, None, None, None, None, None, None, 0, 1, 2, 3, 1, 4, 0, 2, 5, 6, 0, 7, 6, 8, 2, 9, 0, 6, 7, 8, 10, 11, 7, 12, 11, 13, 7, 14, 8, 15, 14, 16, 11, 7, 13, 8, 12, 15, 17, 18, 12, 15, 19, 18, 12, 19, 17, None, 3, 15, 4, 1, 12, 18, 15, 4, 19, 17, 12, 15, 18, 4, 20, 21, 22, 23, 20, 4, 19, 18, 17, 22, 23, 12, 21, 15, 3, 21, 18, 12, 21, 15, 3, 22, 21, 4, 18, 22, 20, 15, 12, 22, 20, 17, 22, 20, 19, 23, 3, 20, 19, 17, 23, 22, 21, 20, 19, 1, 18, 17, 23, 22, 21, 20, 19, 18, 17, None, 15, 12, 3, 4, 6, 0, 9, 2, 5, 1, 12, 3, 2, 5, 6, 9, 0, 12, 4, 3, 23, 22, 21, 20, 23, 3, 19, 18, 21, 23, 20, 3, 2, 4, 19, 5, 18, 6, 21, 9, 23, 0, 22, 12, 15, 17, 6, 12, 4, 24, 0, 12, 15, 6, 17, 4, 3, 25, 9, 4, 5, 26, 3, 9, 25, 4, 17, 0, 12, 5, 26, 15, 6, 3, 24, 9, 20, 27, 21, 9, 19, 28, 1, 9, 20, 21, 27, 19, 2, 29, 23, 19, 18, 2, 27, 1, 23, 9, 20, 19, 21, 29, 25, 24, 18, 4, 17, 2, 27, 0, 12, 1, 23, 5, 26, 9, 20, 15, 19, 6, 28, 22, 3, 28, 22, 9, 28, 24, 2, 3, 28, 24, 6, 28, 24, 1, 22, 28, 29, 2, 3, 28, 29, 6, 28, 29, 9, 15, 0, 29, 2, 3, 28, 24, 29, 4, 22, 21, 20, 22, 21, 27, 20, 22, 18, 23, 27, 20, 21, 22, 25, 19, 27, 20, 21, 25, 22, 19, 17, 26, 21, 25, 27, 22, 20, 19, 18, 17, 23, 26, 28, 21, 24, 25, 29, 27, 22, 20, 19, 18, 17, 23, 26, 28, 24, None]
SLOT_WIDTH = {0: 6, 1: 6, 2: 6, 3: 6, 4: 6, 5: 6, 6: 6, 7: 6, 8: 6, 9: 6, 10: 6, 11: 6, 12: 6, 13: 6, 14: 6, 15: 6, 16: 6, 17: 3, 18: 3, 19: 3, 20: 3, 21: 3, 22: 3, 23: 3, 24: 3, 25: 3, 26: 3, 27: 3, 28: 3, 29: 3}

# ------------------------------------------

BF16 = ml_dtypes.bfloat16
W_CHUNK = 128
N_CHUNK = 256 // W_CHUNK
NL = len(LEAVES)
GRP = W_CHUNK // 32           # conv 32-col groups per chunk
N_UNIT = N_CHUNK * GRP
PE_PER_CHUNK = 20             # 12 T-groups + 4 matmul-triples + 4 backT groups
ACT_PER_CHUNK = 20            # 12 fT copies + 4 cT copies + 4 ostage copies
# per-chunk inc orderings (see engine programs):
#  PE:  T3(g0..3)=1..4, T5=5..8, T7=9..12, then mm0=13,mm1=14,bT0=15,mm2=16,
#       bT1=17,mm3=18,bT2=19,bT3=20
#  ACT: fT3(g0..3)=1..4, fT5=5..8, fT7=9..12, then cT(g)=13+2g, ost(g)=14+2g
MM_INC = [13, 14, 16, 18]
BT_INC = [15, 17, 19, 20]

LAST_RESULTS = None
_NC = None

# ---- instruction pairing (fusion) plan ----
# Independent same-op same-width nodes are fused into one DVE instruction
# with a 3D access pattern ([128, 2, w*16]); all value planes live in one
# SBUF slab so a pair is expressible as offset+stride APs.

_OUT_NODES = {v[0] for v in OUTS.values()}
_WIDTH = [W_CHUNK + hi - lo for (lo, hi) in SPANS]


def _slot_key(vi):
    if vi < NL:
        return ('vin', vi)
    if vi in _OUT_NODES:
        return ('m', vi)
    return ('s', SLOT_OF[vi])


def _key_width(key):
    kind, v = key
    if kind == 'vin':
        return _WIDTH[v] * 16
    if kind == 's':
        return (W_CHUNK + SLOT_WIDTH[v]) * 16
    return _WIDTH[v] * 16


def _all_keys():
    return ([('vin', li) for li in range(NL)]
            + [('s', s) for s in SLOT_WIDTH]
            + [('m', vi) for vi in sorted(_OUT_NODES)])


def _layout_slab(order=None):
    """Slab layout (elem col offsets per slot key) in the given key order."""
    off = {}
    cur = 0
    for key in (order or _all_keys()):
        off[key] = cur
        cur += _key_width(key)
    return off, cur


_SLAB_OFF, _SLAB_W = _layout_slab()
_MAX_STEP = 32000  # signed 16-bit ISA stride field


def _node_offsets(n):
    """(out_off, a_off, b_off) for node n."""
    op, ai, ash, bi, bsh = NODES[n]
    vi = NL + n
    lo = SPANS[vi][0]
    o = _SLAB_OFF[_slot_key(vi)]
    a = _SLAB_OFF[_slot_key(ai)] + (lo + ash - SPANS[ai][0]) * 16
    b = _SLAB_OFF[_slot_key(bi)] + (lo + bsh - SPANS[bi][0]) * 16
    return o, a, b


def _key_cap(key):
    """Slot capacity in elems (how many cols may be written from its base)."""
    return _key_width(key)


def _pair_feasible(n0, n1):
    o0, a0, b0 = _node_offsets(n0)
    o1, a1, b1 = _node_offsets(n1)
    if not (abs(o1 - o0) <= _MAX_STEP and abs(a1 - a0) <= _MAX_STEP
            and abs(b1 - b0) <= _MAX_STEP):
        return False
    # cross-width pair runs at the max width: writes must stay inside each
    # out slot; reads must stay inside the slab
    w16 = max(_WIDTH[NL + n0], _WIDTH[NL + n1]) * 16
    if (_key_cap(_slot_key(NL + n0)) < w16
            or _key_cap(_slot_key(NL + n1)) < w16):
        return False
    for off in (a0, b0, a1, b1):
        if off + w16 > _SLAB_W:
            return False
    return True


_LOOKAHEAD = [24]


def _plan_groups(check_strides=True, allow_cross_width=True,
                 wave_fences=True):
    """Greedy pairing of schedule nodes.  Returns list of groups (1-2 node
    indices each) in emission order."""
    bound = _wave_boundaries()
    out_fence = min(v[0] - NL for v in OUTS.values())
    if wave_fences:
        fences = sorted(set(list(bound.values()) + [out_fence]))
    else:
        fences = [out_fence]

    def seg(n):
        s = 0
        for f in fences:
            if n >= f:
                s += 1
        return s

    n_nodes = len(NODES)
    used = [False] * n_nodes
    groups = []
    LOOKAHEAD = _LOOKAHEAD[0]
    for n in range(n_nodes):
        if used[n]:
            continue
        op, a, ash, b, bsh = NODES[n]
        vi = NL + n
        g = [n]
        used[n] = True
        m7_out = OUTS["m7"][0] - NL
        base_out = n in {v[0] - NL for v in OUTS.values()}
        if n != m7_out:
            w0 = _WIDTH[vi]
            gvals = {vi}
            gout = {_slot_key(vi)}
            skip_vals = set()
            skip_written = set()
            skip_read = set()
            m = n + 1
            cand_cross = None
            while m < min(n_nodes, n + 1 + LOOKAHEAD) and len(g) < 4:
                if used[m]:
                    m += 1
                    continue
                op2, a2, ash2, b2, bsh2 = NODES[m]
                vi2 = NL + m
                sk2 = _slot_key(vi2)
                ok = (op2 == op
                      and seg(m) == seg(n)
                      and m != m7_out
                      and not (base_out and vi2 in _OUT_NODES)
                      and a2 not in gvals and b2 not in gvals
                      and sk2 not in gout
                      and _slot_key(a2) not in gout
                      and _slot_key(b2) not in gout
                      and a2 not in skip_vals and b2 not in skip_vals
                      and sk2 not in skip_written
                      and sk2 not in skip_read
                      and (not check_strides or _pair_feasible(n, m)))
                if ok and _WIDTH[vi2] == w0:
                    if len(g) == 1:
                        accept = True
                    else:
                        # extending beyond a pair requires equal deltas
                        o0, a0, b0 = _node_offsets(g[-2])
                        o1, a1, b1 = _node_offsets(g[-1])
                        o2, a2o, b2o = _node_offsets(m)
                        accept = (o2 - o1 == o1 - o0
                                  and a2o - a1 == a1 - a0
                                  and b2o - b1 == b1 - b0
                                  and _WIDTH[NL + g[0]] == w0)
                    if accept:
                        g.append(m)
                        used[m] = True
                        gvals.add(vi2)
                        gout.add(sk2)
                        cand_cross = None
                        m += 1
                        continue
                if ok and allow_cross_width and cand_cross is None and len(g) == 1:
                    cand_cross = m
                skip_vals.add(vi2)
                skip_written.add(sk2)
                skip_read.add(_slot_key(a2))
                skip_read.add(_slot_key(b2))
                m += 1
            if cand_cross is not None and len(g) == 1:
                g.append(cand_cross)
                used[cand_cross] = True
        groups.append(g)
    return groups, bound, fences




# conv plane order = completion order in the schedule (m3 first, m7 last);
# wblk index = weight block column in wpack ([w7|w3|w5|ident])
PLANES = (("m3", 1), ("m5", 2), ("m7", 0))

# leaf-load waves for chunk 0
WAVE = {
    "A1": [li for li, dv in enumerate(LEAVES) if dv in (-1, 0)],
    "A2": [li for li, dv in enumerate(LEAVES) if dv == 1],
    "B": [li for li, dv in enumerate(LEAVES) if dv in (-2, 2)],
    "C": [li for li, dv in enumerate(LEAVES) if dv in (-3, 3)],
}


def _wave_boundaries():
    """First op index needing each wave (None -> op 0)."""
    bound = {}
    for w in ("A2", "B", "C"):
        ws = set(WAVE[w])
        bound[w] = next(n for n, (_o, ai, _as, bi, _bs) in enumerate(NODES)
                        if ai in ws or bi in ws)
    return bound


def _affinity_layout():
    """Order slot keys so that (unconstrained-)paired ops' slots are close,
    maximizing pairs that fit the 16-bit AP stride field."""
    groups0, _, _ = _plan_groups(check_strides=False, allow_cross_width=False)
    import collections
    aff = collections.Counter()
    for g in groups0:
        if len(g) != 2:
            continue
        n0, n1 = g
        _o0, a0, _s, b0, _s2 = NODES[n0]
        _o1, a1, _s3, b1, _s4 = NODES[n1]
        for x, y in (((NL + n0), (NL + n1)), (a0, a1), (b0, b1)):
            kx, ky = _slot_key(x), _slot_key(y)
            if kx != ky:
                aff[frozenset((kx, ky))] += 1
    keys = _all_keys()
    deg = collections.Counter()
    for e, w in aff.items():
        for k in e:
            deg[k] += w
    placed = []
    remaining = set(keys)
    cur = max(remaining, key=lambda k: deg.get(k, 0))
    placed.append(cur)
    remaining.discard(cur)
    while remaining:
        # affinity to the last few placed keys, recency-weighted
        best, bestw = None, -1.0
        for k in remaining:
            w = 0.0
            for d, p in enumerate(reversed(placed[-14:])):
                w += aff.get(frozenset((k, p)), 0) / (1.0 + 0.3 * d)
            if w > bestw:
                best, bestw = k, w
        placed.append(best)
        remaining.discard(best)
    return placed


_SLAB_OFF, _SLAB_W = _layout_slab(_affinity_layout())
def _best_plan(wave_fences):
    best = None
    for la in (16, 24, 32, 40, 56):
        _LOOKAHEAD[0] = la
        for acw in (False,):
            r = _plan_groups(allow_cross_width=acw, wave_fences=wave_fences)
            if best is None or len(r[0]) < len(best[0]):
                best = r
    return best


_GROUPS, _BOUND, _FENCES = _best_plan(True)
_GROUPS1 = _best_plan(False)[0]

# tail-split feasibility: last chunk's final two groups must be exactly
# {m7's two producers} and {m7}, with zero relative column offsets
def _tail_split_ok():
    m7 = OUTS['m7'][0] - NL
    pa, pb = NODES[m7][1] - NL, NODES[m7][3] - NL
    if NODES[m7][2] != 0 or NODES[m7][4] != 0:
        return False, []
    lom = SPANS[NL + m7][0]
    if (SPANS[NL + pa][0] != lom or SPANS[NL + pb][0] != lom
            or _WIDTH[NL + pa] != 128 or _WIDTH[NL + pb] != 128
            or _WIDTH[NL + m7] != 128):
        return False, []
    if len(_GROUPS1) < 2 or _GROUPS1[-1] != [m7]:
        return False, []
    if sorted(_GROUPS1[-2]) != sorted([pa, pb]):
        return False, []
    return True, _GROUPS1[-2]


_TAIL_SPLIT_OK, _TAIL_PAIR = _tail_split_ok()
# emission-order index of the last group touching a leaf (s_cols attach)
_LAST_LEAF_GROUP = max(
    gi for gi, g in enumerate(_GROUPS)
    if any(NODES[n][1] < NL or NODES[n][3] < NL for n in g))


def _build_nc():
    nc = bass.Bass()
    dt = mybir.dt
    x = nc.declare_dram_parameter("x", [134, 262 * 16], dt.bfloat16, False)
    wpack = nc.declare_dram_parameter("wpack", [128, 4 * 128], dt.bfloat16, False)
    outd = nc.declare_dram_parameter("out", [128, 256 * 16], dt.bfloat16, True)

    out_nodes = {v[0] for v in OUTS.values()}
    width = [W_CHUNK + hi - lo for (lo, hi) in SPANS]
    bound = _wave_boundaries()
    last_leaf_op = max(n for n, (_o, ai, _as, bi, _bs) in enumerate(NODES)
                       if ai < NL or bi < NL)
    # op index of each output write, and its conv plane position (0..2)
    out_op = {}   # op index -> (plane_pos, sem_index)
    for pos, (key, _wb) in enumerate(PLANES):
        vi, _osh = OUTS[key]
        out_op[vi - NL] = pos

    ctx = ExitStack()
    _uid = [0]

    def sb(shape, d, name=None):
        _uid[0] += 1
        return ctx.enter_context(
            nc.sbuf_tensor(name or f"t{_uid[0]}", shape, d))

    def ps(shape, d, name=None):
        _uid[0] += 1
        return ctx.enter_context(
            nc.psum_tensor(name or f"p{_uid[0]}", shape, d))

    def sem(name):
        return ctx.enter_context(nc.semaphore(name))

    with ctx:
        wpack_t = sb([128, 512], dt.bfloat16)
        # one slab holds every value plane (leaves, slots, m-planes) so a
        # fused pair of ops is expressible as a 3D offset+stride AP
        slab_off = _SLAB_OFF
        SLAB_W = _SLAB_W
        slab = sb([128, SLAB_W], dt.bfloat16)

        def slab_ap(col, w, r0=0, nr=128):
            """2D AP: [nr partitions, w elems] at column offset col."""
            return slab[r0:r0 + nr, col:col + w]

        _slab_h = slab[:, 0:1].tensor

        def slab_apN(col0, delta, k, w):
            """3D AP: [128, k, w] members at col0 + i*delta."""
            return BassAP(tensor=_slab_h, offset=col0,
                          ap=[[SLAB_W, 128], [delta, k], [1, w]])

        vin_off = [slab_off[('vin', li)] for li in range(NL)]
        m_off = {vi: slab_off[('m', vi)] for vi in out_nodes}
        fT = [[sb([128, 512], dt.bfloat16) for _g in range(GRP)]
              for _p in range(3)]
        cT = [sb([128, 512], dt.bfloat16) for _ in range(2)]
        ostage = [sb([128, 512], dt.bfloat16) for _ in range(3)]
        # plane 2 (m7) is tail-critical: double-buffer it; planes 0/1 single
        psumT = [[ps([128, 512], dt.bfloat16)] for _p in range(2)]
        psumT.append([ps([128, 512], dt.bfloat16) for _g in range(2)])
        psumC = [ps([128, 512], dt.float32) for _ in range(2)]
        psumB = [ps([128, 512], dt.bfloat16) for _ in range(2)]

        s_wave = {w: sem(f"s_{w}") for w in WAVE}
        s_h = sem("s_h")    # A1 col piece [0,60)
        s_h2 = sem("s_h2")  # A1 col piece [60,100)
        s_in1 = sem("s_in1")
        s_cols = sem("s_cols")   # chunk column-stage done (vin consumed)
        s_w = sem("s_w")
        s_m = [sem(f"s_m{i}") for i in range(3)]   # per-plane completion
        s_pe = sem("s_pe")
        s_act = sem("s_act")
        s_out = sem("s_out")

        def leaf_dma(sync, ci, li, row0, nrows, semh):
            lo, hi = SPANS[li]
            dv = LEAVES[li]
            c0 = ci * W_CHUNK + 3 + lo
            sync.dma_start(
                out=slab_ap(vin_off[li], width[li] * 16, row0, nrows),
                in_=x[dv + 3 + row0:dv + 3 + row0 + nrows,
                      c0 * 16:(c0 + width[li]) * 16],
            ).then_inc(semh, 16)

        with nc.Block() as block:

            @block.sync
            def _(sync):
                # wave A1 loads as three column pieces under dedicated
                # semaphores (full counts only — race-free) so the first
                # two (shift-0) ops can start before the full leaf width
                # arrives and the critical last piece is small
                k = 0
                for cs, ce, semh in ((0, 60, s_h), (60, 100, s_h2),
                                     (100, None, s_wave["A1"])):
                    for li in WAVE["A1"]:
                        k += 1
                        if k % 2 == 0:
                            continue  # odd pieces issued by ACT
                        lo, hi = SPANS[li]
                        dv = LEAVES[li]
                        c0 = 3 + lo
                        e = ce if ce is not None else width[li]
                        sync.dma_start(
                            out=slab_ap(vin_off[li] + cs * 16,
                                        (e - cs) * 16),
                            in_=x[dv + 3:dv + 3 + 128,
                                  (c0 + cs) * 16:(c0 + e) * 16],
                        ).then_inc(semh, 16)
                for w, split in (("A2", 4), ("B", 2), ("C", 2)):
                    step = 128 // split
                    for r in range(split):
                        for li in WAVE[w]:
                            k += 1
                            if k % 2 == 0:
                                continue
                            leaf_dma(sync, 0, li, r * step, step, s_wave[w])
                sync.dma_start(out=wpack_t[:], in_=wpack[:]).then_inc(s_w, 16)
                sync.wait_ge(s_cols, 1)
                for li in range(NL):
                    leaf_dma(sync, 1, li, 0, 128, s_in1)
                for u in range(N_UNIT):
                    ci, g = divmod(u, GRP)
                    sync.wait_ge(s_act, ACT_PER_CHUNK * ci + 14 + 2 * g)
                    o0 = (ci * W_CHUNK + g * 32) * 16
                    sync.dma_start(
                        out=outd[0:128, o0:o0 + 512], in_=ostage[u % 3][:]
                    ).then_inc(s_out, 16)

            @block.vector
            def _(vector):
                # value vi lives at slab col base_off(vi); its col 0 is span-lo
                def val_off(vi):
                    return slab_off[_slot_key(vi)]

                def opnd_off(x, lo, sh):
                    """abs slab elem offset of operand value x read at
                    (node-span-lo + shift sh)."""
                    xlo = SPANS[x][0]
                    return val_off(x) + (lo + sh - xlo) * 16


                out_fence = min(out_op)
                m7_op = OUTS["m7"][0] - NL
                for ci in range(N_CHUNK):
                    if ci == 0:
                        for cs, ce, semh in ((0, 60, s_h),
                                             (60, 100, s_h2),
                                             (100, None, s_wave["A1"])):
                            vector.wait_ge(semh, 16 * len(WAVE["A1"]))
                            for g in _GROUPS[:2]:
                                assert len(g) == 1
                                nn0 = g[0]
                                opn, ai, ash, bi, bsh = NODES[nn0]
                                assert ash == 0 and bsh == 0
                                vi = NL + nn0
                                lo = SPANS[vi][0]
                                e = ce if ce is not None else width[vi]
                                vector.tensor_tensor(
                                    slab_ap(val_off(vi) + cs * 16,
                                            (e - cs) * 16),
                                    slab_ap(opnd_off(ai, lo, ash) + cs * 16,
                                            (e - cs) * 16),
                                    slab_ap(opnd_off(bi, lo, bsh) + cs * 16,
                                            (e - cs) * 16),
                                    op=(mybir.AluOpType.min
                                        if opn == "min"
                                        else mybir.AluOpType.max),
                                )
                    else:
                        vector.wait_ge(s_in1, 16 * NL)
                    waved = set()
                    fenced = False
                    glist = _GROUPS if ci == 0 else _GROUPS1
                    for gi, g in enumerate(glist):
                        if ci == 0 and gi < 2:
                            continue
                        n0 = g[0]
                        if ci == 0:
                            for w, split in (("A2", 4), ("B", 2), ("C", 2)):
                                if w not in waved and n0 >= bound[w]:
                                    vector.wait_ge(
                                        s_wave[w],
                                        16 * split * len(WAVE[w]))
                                    waved.add(w)
                        elif not fenced and n0 >= out_fence:
                            # single-buffered m-planes: one conservative wait
                            # (covers all three planes' WAR) at the earliest
                            # output write instead of three 1us bubbles
                            vector.wait_ge(
                                s_pe, PE_PER_CHUNK * (ci - 1) + 12)
                            fenced = True
                        op = NODES[n0][0]
                        alu = (mybir.AluOpType.min if op == "min"
                               else mybir.AluOpType.max)
                        # last chunk: interleave column-halves of the final
                        # m7 producers with the m7 quarters so the conv tail
                        # starts ~2 ops earlier (all three have span-lo 3 and
                        # shift 0, so columns align exactly)
                        if (ci == N_CHUNK - 1 and gi == len(glist) - 2
                                and _TAIL_SPLIT_OK
                                and sorted(g) == sorted(_TAIL_PAIR)):
                            p0, p1 = g
                            oo, aa, bb = [], [], []
                            for n_ in g:
                                _o, a_, as_, b_, bs_ = NODES[n_]
                                v_ = NL + n_
                                lo_ = SPANS[v_][0]
                                oo.append(val_off(v_))
                                aa.append(opnd_off(a_, lo_, as_))
                                bb.append(opnd_off(b_, lo_, bs_))
                            opm, am, asm, bm, bsm = NODES[m7_op]
                            vm = NL + m7_op
                            lom = SPANS[vm][0]
                            alum = (mybir.AluOpType.min if opm == "min"
                                    else mybir.AluOpType.max)
                            for h in range(2):
                                ho = h * 64 * 16
                                vector.tensor_tensor(
                                    slab_apN(oo[0] + ho, oo[1] - oo[0],
                                             2, 64 * 16),
                                    slab_apN(aa[0] + ho, aa[1] - aa[0],
                                             2, 64 * 16),
                                    slab_apN(bb[0] + ho, bb[1] - bb[0],
                                             2, 64 * 16),
                                    op=alu,
                                )
                                for q in (2 * h, 2 * h + 1):
                                    qo = q * 32 * 16
                                    vector.tensor_tensor(
                                        slab_ap(val_off(vm) + qo, 32 * 16),
                                        slab_ap(opnd_off(am, lom, asm) + qo,
                                                32 * 16),
                                        slab_ap(opnd_off(bm, lom, bsm) + qo,
                                                32 * 16),
                                        op=alum,
                                    ).then_inc(s_m[2], 1)
                            continue
                        if (ci == N_CHUNK - 1 and gi == len(glist) - 1
                                and _TAIL_SPLIT_OK):
                            continue  # m7 already emitted interleaved above
                        if len(g) == 1 and n0 == m7_op:
                            # final m7 op: emit in 4 col-quarters, inc
                            # s_m[2] per quarter so PE transposes of group
                            # g start before the full plane is done
                            op_, ai, ash, bi, bsh = NODES[n0]
                            vi = NL + n0
                            lo = SPANS[vi][0]
                            for q in range(4):
                                qo = q * 32 * 16
                                vector.tensor_tensor(
                                    slab_ap(val_off(vi) + qo, 32 * 16),
                                    slab_ap(opnd_off(ai, lo, ash) + qo,
                                            32 * 16),
                                    slab_ap(opnd_off(bi, lo, bsh) + qo,
                                            32 * 16),
                                    op=alu,
                                ).then_inc(s_m[2], 1)
                            continue
                        if len(g) == 1:
                            op_, ai, ash, bi, bsh = NODES[n0]
                            vi = NL + n0
                            lo = SPANS[vi][0]
                            wdt = width[vi]
                            inst = vector.tensor_tensor(
                                slab_ap(val_off(vi), wdt * 16),
                                slab_ap(opnd_off(ai, lo, ash), wdt * 16),
                                slab_ap(opnd_off(bi, lo, bsh), wdt * 16),
                                op=alu,
                            )
                        else:
                            oo, aa, bb = [], [], []
                            for n_ in g:
                                _o, a_, as_, b_, bs_ = NODES[n_]
                                v_ = NL + n_
                                lo_ = SPANS[v_][0]
                                oo.append(val_off(v_))
                                aa.append(opnd_off(a_, lo_, as_))
                                bb.append(opnd_off(b_, lo_, bs_))
                            w16 = max(width[NL + n_] for n_ in g) * 16
                            k = len(g)
                            inst = vector.tensor_tensor(
                                slab_apN(oo[0], oo[1] - oo[0], k, w16),
                                slab_apN(aa[0], aa[1] - aa[0], k, w16),
                                slab_apN(bb[0], bb[1] - bb[0], k, w16),
                                op=alu,
                            )
                        for n in g:
                            if n in out_op:
                                inst.then_inc(s_m[out_op[n]], 1)
                        if ci == 0 and gi == _LAST_LEAF_GROUP:
                            inst.then_inc(s_cols, 1)
                    if ci == N_CHUNK - 1:
                        # DVE is idle after its last op: take over the tail
                        # cT/ostage copies from ACT to shorten the epilogue
                        pb = PE_PER_CHUNK * ci
                        for g in range(GRP):
                            u = ci * GRP + g
                            # cT double-buffered: WAR is bT(g-2), and
                            # max(MM_INC[g], BT_INC[g-2]) == MM_INC[g]
                            vector.wait_ge(s_pe, pb + MM_INC[g])
                            vector.tensor_copy(
                                cT[g % 2][:],
                                psumC[g % 2][:]).then_inc(s_act, 1)
                            vector.wait_ge(s_pe, pb + BT_INC[g])
                            if u >= 3:
                                vector.wait_ge(s_out, 16 * (u - 2))
                            vector.tensor_copy(
                                ostage[u % 3][:],
                                psumB[g % 2][:]).then_inc(s_act, 1)

            @block.tensor
            def _(tensor):
                tensor.wait_ge(s_w, 16)
                for ci in range(N_CHUNK):
                    pb = PE_PER_CHUNK * ci
                    ab = ACT_PER_CHUNK * ci
                    # transpose phases, plane by plane (completion order)
                    for p, (key, _wb) in enumerate(PLANES):
                        if p == 2 and ci == N_CHUNK - 1:
                            mm_a(0)
                            mm_a(1)
                        vi, osh = OUTS[key]
                        moff = m_off[vi]
                        start = osh - SPANS[vi][0]
                        if p != 2:
                            tensor.wait_ge(s_m[p], ci + 1)
                        nb = len(psumT[p])
                        for g in range(GRP):
                            if p == 2:
                                # m7 arrives in col-quarters (4 incs/chunk)
                                tensor.wait_ge(s_m[2], 4 * ci + g + 1)
                            # psumT[p][g%nb] WAR: fT_p copy nb groups back
                            war = ab + 4 * p + g + 1 - nb
                            if war >= 1:
                                tensor.wait_ge(s_act, war)
                            for blk in range(4):
                                col0 = (start + g * 32 + blk * 8) * 16
                                inst = nc.tensor.transpose(
                                    psumT[p][g % nb][:, blk * 128:(blk + 1) * 128],
                                    slab_ap(moff + col0, 128),
                                    wpack_t[:, 384:512],
                                )
                            inst.then_inc(s_pe, 1)
                    # last chunk: m3+m5 products for groups 0/1 issue
                    # mid-chunk (before the m7-gated T7 transposes), leaving
                    # only the m7 matmul on the tail critical path
                    def mm_a(g):
                        # fT5(g) inc (ab+5+g) implies fT3(g) (ab+1+g)
                        tensor.wait_ge(s_act, ab + 5 + g)
                        if g >= 2:
                            tensor.wait_ge(s_act, ab + 9 + 2 * g)
                        for p, (_key, wb) in enumerate(PLANES[:2]):
                            nc.tensor.matmul(
                                psumC[g % 2][:],
                                wpack_t[:, wb * 128:(wb + 1) * 128],
                                fT[p][g][:], start=(p == 0), stop=False,
                            )

                    def mm_b(g):
                        tensor.wait_ge(s_act, ab + 9 + g)
                        _key, wb = PLANES[2]
                        nc.tensor.matmul(
                            psumC[g % 2][:],
                            wpack_t[:, wb * 128:(wb + 1) * 128],
                            fT[2][g][:], start=False, stop=True,
                        ).then_inc(s_pe, 1)

                    # matmul triples and back-transposes, interleaved
                    def mm(g):
                        tensor.wait_ge(s_act, ab + 9 + g)
                        if g >= 2:
                            tensor.wait_ge(s_act, ab + 9 + 2 * g)
                        for p, (_key, wb) in enumerate(PLANES):
                            inst = nc.tensor.matmul(
                                psumC[g % 2][:],
                                wpack_t[:, wb * 128:(wb + 1) * 128],
                                fT[p][g][:], start=(p == 0), stop=(p == 2),
                            )
                        inst.then_inc(s_pe, 1)

                    def bT(g):
                        tensor.wait_ge(s_act, ab + 13 + 2 * g)
                        for blk in range(4):
                            inst = nc.tensor.transpose(
                                psumB[g % 2][:, blk * 128:(blk + 1) * 128],
                                cT[g % 2][:, blk * 128:(blk + 1) * 128],
                                wpack_t[:, 384:512],
                            )
                        inst.then_inc(s_pe, 1)

                    if ci == N_CHUNK - 1:
                        mm_b(0); mm_b(1); bT(0); mm(2); bT(1); mm(3)
                        bT(2); bT(3)
                    else:
                        mm(0); mm(1); bT(0); mm(2); bT(1); mm(3)
                        bT(2); bT(3)

            @block.scalar
            def _(scalar):
                k = 0
                for cs, ce, semh in ((0, 60, s_h), (60, 100, s_h2),
                                     (100, None, s_wave["A1"])):
                    for li in WAVE["A1"]:
                        k += 1
                        if k % 2 == 1:
                            continue  # even pieces issued by SP
                        lo, hi = SPANS[li]
                        dv = LEAVES[li]
                        c0 = 3 + lo
                        e = ce if ce is not None else width[li]
                        scalar.dma_start(
                            out=slab_ap(vin_off[li] + cs * 16,
                                        (e - cs) * 16),
                            in_=x[dv + 3:dv + 3 + 128,
                                  (c0 + cs) * 16:(c0 + e) * 16],
                        ).then_inc(semh, 16)
                for w, split in (("A2", 4), ("B", 2), ("C", 2)):
                    step = 128 // split
                    for r in range(split):
                        for li in WAVE[w]:
                            k += 1
                            if k % 2 == 1:
                                continue
                            leaf_dma(scalar, 0, li, r * step, step,
                                     s_wave[w])
                for ci in range(N_CHUNK):
                    pb = PE_PER_CHUNK * ci
                    for p in range(3):
                        nb = len(psumT[p])
                        for g in range(GRP):
                            scalar.wait_ge(s_pe, pb + 4 * p + g + 1)
                            scalar.copy(fT[p][g][:],
                                        psumT[p][g % nb][:]).then_inc(s_act, 1)
                    if ci == N_CHUNK - 1:
                        continue  # tail copies run on DVE (idle by then)
                    for g in range(GRP):
                        u = ci * GRP + g
                        # cT double-buffered: WAR is bT(g-2), and
                        # max(MM_INC[g], BT_INC[g-2]) == MM_INC[g]
                        scalar.wait_ge(s_pe, pb + MM_INC[g])
                        scalar.copy(cT[g % 2][:],
                                    psumC[g % 2][:]).then_inc(s_act, 1)
                        scalar.wait_ge(s_pe, pb + BT_INC[g])
                        if u >= 3:
                            scalar.wait_ge(s_out, 16 * (u - 2))
                        scalar.copy(ostage[u % 3][:],
                                    psumB[g % 2][:]).then_inc(s_act, 1)

    return nc


def _get_nc():
    global _NC
    if _NC is None:
        _NC = _build_nc()
    return _NC


def kernel(x, high_kernel):
    global LAST_RESULTS
    if not os.environ.get("TRN_TRACE"):
        # tracing needs an NTFF hook this image may not have; make sure a
        # stray BASS_TRACE in the environment can't activate it
        os.environ.setdefault("BASS_NEVER_TRACE", "1")
    x = np.asarray(x, dtype=np.float32)
    hk = np.asarray(high_kernel, dtype=np.float32)
    B, H, W, C = x.shape
    assert (B, H, W, C) == (4, 256, 256, 16)

    xp = np.zeros((B, H + 6, W + 6, C), np.float32)
    xp[:, 3:-3, 3:-3, :] = x
    xbf = xp.astype(BF16)

    wm = hk[0, 0]  # [48, 16] (c, o)
    wpack = np.zeros((128, 512), np.float32)
    for p, sl in enumerate((slice(0, 16), slice(16, 32), slice(32, 48))):
        sub = wm[sl]
        for wl in range(8):
            wpack[wl * 16:(wl + 1) * 16,
                  p * 128 + wl * 16:p * 128 + (wl + 1) * 16] = sub
    wpack[:, 384:512] = np.eye(128)
    wpack = wpack.astype(BF16)

    in_maps = []
    for core in range(8):
        b, hh = divmod(core, 2)
        sh = np.ascontiguousarray(
            xbf[b, hh * 128: hh * 128 + 134].reshape(134, 262 * 16))
        in_maps.append({"x": sh, "wpack": wpack})

    nc = _get_nc()
    res = run_bass_kernel_spmd(
        nc, in_maps, core_ids=list(range(8)),
        trace=bool(os.environ.get("TRN_TRACE")),
    )
    LAST_RESULTS = res

    outf = np.empty((4, 256, 256, 16), np.float32)
    for core in range(8):
        b, hh = divmod(core, 2)
        o = np.asarray(res.results[core]["out"]).astype(np.float32)
        outf[b, hh * 128:(hh + 1) * 128] = o.reshape(128, 256, 16)
    return outf

